# revision 1
# baseline (speedup 1.0000x reference)
"""Trainium2 Bass kernel for MultiHeadedAttentionSANM.

Per-core (data-parallel over batch, 8 cores, B=1 each):
  - qkv^T = (x @ Wqkv)^T on PE (float32r): q^T and full v^T (FSMN needs all
    tokens); k^T and a second v^T are computed only for the ~50% of tokens
    with mask=1, gathered host-side into a compact x_c (TK tokens).
  - FSMN: depthwise conv over time in (d, t) layout on DVE (f32), in place
    on v^T with partial-width taps; mask-muls on GPSIMD.
  - attention: scores computed transposed (compact keys on partitions) so the
    exp output feeds the ctx matmul directly as the rhs stream; masked/padded
    keys get a -30000 exp bias. Softmax denominator Z via a ones-weight PE
    pass; normalization is delayed all the way to the out-proj epilogue
    (per-head PSUM + per-partition 1/Z scalars).
"""

import os
import sys

for _p in ("/opt/trn_rl_repo", "/root/.axon_site/_ro/trn_rl_repo"):
    if os.path.isdir(_p) and _p not in sys.path:
        sys.path.append(_p)

from contextlib import ExitStack

import numpy as np

import concourse.bass as bass
import concourse.mybir as mybir
import concourse.tile as tile
from concourse import bacc
from concourse import bass_utils
from concourse.masks import make_identity

T, D, H, DK, KS, PAD = 2048, 512, 4, 128, 11, 5
NCORES = 8
NT = T // 128          # 16 t-blocks of 128
NC = D // 128          # 4 d-chunks of 128
SCALE = float(DK) ** -0.5
MASK_NEG = -30000.0

F32 = mybir.dt.float32
F32R = mybir.dt.float32r
BF16 = mybir.dt.bfloat16
AF = mybir.ActivationFunctionType
OP = mybir.AluOpType

QKV_DT = os.environ.get("SANM_QKV_DT", "f32r")   # f32r | f32
ATT_DT = os.environ.get("SANM_ATT_DT", "bf16")   # bf16 | f32
REPS = int(os.environ.get("SANM_REPS", "1"))     # timing: repeat body in one NEFF
NORM = os.environ.get("SANM_NORM", "psum")       # psum | inplace
FSMN_DT = os.environ.get("SANM_FSMN_DT", "f32")  # f32 | bf16


def _bcast_vec(ap, nrows):
    """Broadcast a flat [N] DRAM AP across partitions -> [nrows, N]."""
    return bass.AP(tensor=ap.tensor, offset=ap.offset, ap=[[0, nrows]] + list(ap.ap))


def _tiles(total, step=512):
    out, p = [], 0
    while p < total:
        n = min(step, total - p)
        rem = total - p - n
        if 0 < rem < 256:  # avoid <256-wide f32r tails (4 cyc/row penalty)
            n = (n + rem) // 2
            n = (n + 127) // 128 * 128
        out.append((p, n))
        p += n
    return out


def build_kernel_body(tc, aps, TK, rep=0):
    nc = tc.nc
    x_d, mask_d, xc_d, cbias_d, wqkv_d, bqkv_d, wout_d, bout_d, fw_d, out_d = aps
    R = f"r{rep}_" if rep else ""
    TKC = TK // 128  # compact key chunks

    att_store = F32 if ATT_DT == "f32" else BF16
    qkv_store = F32R if QKV_DT == "f32r" else F32

    stack = ExitStack()
    consts = stack.enter_context(tc.tile_pool(name=R + "consts", bufs=1))
    work = stack.enter_context(tc.tile_pool(name=R + "work", bufs=2))
    ps = stack.enter_context(tc.tile_pool(name=R + "ps", bufs=1, space="PSUM"))
    dram = stack.enter_context(tc.tile_pool(name=R + "dram", bufs=2, space="DRAM"))

    # p_main holds all long-lived tensors (whole kernel); p_x nests inside it
    # (LIFO) and is released after the qkv matmuls to reclaim x^T space.
    main_cm = tc.tile_pool(name=R + "p_main", bufs=1)
    x_cm = tc.tile_pool(name=R + "p_x", bufs=1)
    p_main = main_cm.__enter__()
    p_x = x_cm.__enter__()

    # ---------------- constants ----------------
    ident = consts.tile([128, 128], F32, name="ident", tag="ident")
    make_identity(nc, ident)
    ident_b = consts.tile([128, 128], att_store, name="ident_b", tag="ident_b")
    nc.vector.tensor_copy(ident_b, ident)

    ones_att = consts.tile([128, 1], att_store, name="ones_att", tag="ones_att")
    nc.vector.memset(ones_att, 1.0)

    # compact-key exp bias (0 valid / -30000 padded), as columns (128, TKC)
    mbias = consts.tile([128, TKC], F32, name="mbias", tag="mbias")
    nc.sync.dma_start(out=mbias, in_=cbias_d.rearrange("(c p) -> p c", p=128))

    # mask broadcast across partitions (128, T) bf16 (exact for 0/1), for FSMN
    mrow = consts.tile([128, T], BF16, name="mrow", tag="mrow")
    nc.gpsimd.dma_start(out=mrow, in_=_bcast_vec(mask_d, 128))

    # biases as per-partition columns
    bq = consts.tile([128, 12], F32, name="bq", tag="bq")
    nc.sync.dma_start(out=bq, in_=bqkv_d.rearrange("(c p) -> p c", p=128))
    bo = consts.tile([128, NC], F32, name="bo", tag="bo")
    nc.sync.dma_start(out=bo, in_=bout_d.rearrange("(c p) -> p c", p=128))

    # fsmn weights (128, NC, KS); center tap += 1 (folds the residual)
    wadj = consts.tile([128, NC, KS], F32, name="wadj", tag="wadj")
    nc.scalar.dma_start(out=wadj, in_=fw_d.rearrange("(c p) o k -> p c (o k)", p=128))
    if FSMN_DT == "f32":
        nc.vector.tensor_scalar_add(
            wadj[:, :, PAD : PAD + 1], wadj[:, :, PAD : PAD + 1], 1.0
        )

    # Wout (128, NC, 512) in attention dtype (staged through a work tile)
    wo = consts.tile([128, NC, D], att_store, name="wo", tag="wo")
    for c in range(NC):
        wos = work.tile([128, D], F32, name="wos", tag="wos", bufs=2)
        nc.scalar.dma_start(out=wos, in_=wout_d[c * 128 : (c + 1) * 128, :])
        nc.vector.tensor_copy(wo[:, c, :], wos)

    # ---------------- x^T and xc^T (PE transposes) ----------------
    xT = [p_x.tile([128, T], qkv_store, name=f"xT{c}", tag=f"xT{c}") for c in range(NC)]
    for ti in range(NT):
        xn = work.tile([128, D], F32, name="xnat", tag="xnat", bufs=3)
        (nc.sync if ti % 2 == 0 else nc.scalar).dma_start(
            out=xn, in_=x_d[ti * 128 : (ti + 1) * 128, :]
        )
        for c in range(NC):
            tp = ps.tile([128, 128], F32, name="tp", tag="s", bufs=4)
            nc.tensor.transpose(tp, xn[:, c * 128 : (c + 1) * 128], ident)
            dst = xT[c][:, ti * 128 : (ti + 1) * 128]
            if (ti + c) % 2 == 0:
                nc.vector.tensor_copy(dst, tp)
            else:
                nc.scalar.copy(dst, tp)
    xcT = [
        p_x.tile([128, TK], qkv_store, name=f"xcT{c}", tag=f"xcT{c}")
        for c in range(NC)
    ]
    for ti in range(TKC):
        xn = work.tile([128, D], F32, name="xnat", tag="xnat", bufs=3)
        nc.scalar.dma_start(out=xn, in_=xc_d[ti * 128 : (ti + 1) * 128, :])
        for c in range(NC):
            tp = ps.tile([128, 128], F32, name="tpc", tag="s", bufs=4)
            nc.tensor.transpose(tp, xn[:, c * 128 : (c + 1) * 128], ident)
            dst = xcT[c][:, ti * 128 : (ti + 1) * 128]
            if (ti + c) % 2 == 0:
                nc.vector.tensor_copy(dst, tp)
            else:
                nc.scalar.copy(dst, tp)

    # ---------------- qkv^T = (x @ Wqkv)^T ----------------
    # q on full tokens; k only compact; v full (FSMN) and compact (attention)
    qT = [p_main.tile([128, T], att_store, name=f"qT{h}", tag=f"qT{h}") for h in range(H)]
    kTc = [p_main.tile([128, TK], att_store, name=f"kTc{h}", tag=f"kTc{h}") for h in range(H)]
    vT = [p_main.tile([128, T], F32, name=f"vT{c}", tag=f"vT{c}") for c in range(NC)]
    vcT = [p_main.tile([128, TK], att_store, name=f"vcT{c}", tag=f"vcT{c}") for c in range(NC)]

    def project(f, src, tspans, sink):
        """psum[128, n] = Wqkv[:, f-block].T @ src over d-chunks, then sink."""
        wqf = work.tile([128, NC, 128], qkv_store, name="wqf", tag="wqf", bufs=3)
        wqf_src = wqkv_d[:, f * 128 : (f + 1) * 128].rearrange("(c p) f -> p c f", p=128)
        if QKV_DT == "f32r":
            wqf_src = wqf_src.bitcast(F32R)
        (nc.scalar if f % 2 else nc.sync).dma_start(out=wqf, in_=wqf_src)
        for t0, n in tspans:
            mm = ps.tile([128, 512], F32, name="mmq", tag="s", bufs=4)
            for dc in range(NC):
                nc.tensor.matmul(
                    mm[:, :n],
                    wqf[:, dc, :],
                    src[dc][:, t0 : t0 + n],
                    start=(dc == 0),
                    stop=(dc == NC - 1),
                )
            sink(mm, t0, n)

    for f in range(4):  # q: full tokens -> qT (bf16), bias via ACT
        def sink_q(mm, t0, n, f=f):
            nc.scalar.activation(
                qT[f][:, t0 : t0 + n], mm[:, :n], AF.Identity,
                bias=bq[:, f : f + 1], scale=1.0,
            )
        project(f, xT, _tiles(T), sink_q)
    for f in range(4, 8):  # k: compact tokens -> kTc
        def sink_k(mm, t0, n, f=f):
            nc.scalar.activation(
                kTc[f - 4][:, t0 : t0 + n], mm[:, :n], AF.Identity,
                bias=bq[:, f : f + 1], scale=1.0,
            )
        project(f, xcT, _tiles(TK), sink_k)
    for f in range(8, 12):  # v full tokens (FSMN), f32, bias via DVE
        def sink_v(mm, t0, n, f=f):
            nc.vector.tensor_scalar_add(
                vT[f - 8][:, t0 : t0 + n], mm[:, :n], bq[:, f : f + 1]
            )
        project(f, xT, _tiles(T), sink_v)
    for f in range(8, 12):  # v compact tokens (attention), att dtype
        def sink_vc(mm, t0, n, f=f):
            nc.scalar.activation(
                vcT[f - 8][:, t0 : t0 + n], mm[:, :n], AF.Identity,
                bias=bq[:, f : f + 1], scale=1.0,
            )
        project(f, xcT, _tiles(TK), sink_vc)
    x_cm.__exit__(None, None, None)  # frees xT, xcT

    # ------------- compact v natural (PE transposes of vcT, att dtype) --------
    vh = [
        p_main.tile([128, TKC, 128], att_store, name=f"vh{h}", tag=f"vh{h}")
        for h in range(H)
    ]
    for h in range(H):
        for jc in range(TKC):
            tp = ps.tile([128, 128], att_store, name="tpv", tag="s", bufs=4)
            nc.tensor.transpose(tp, vcT[h][:, jc * 128 : (jc + 1) * 128], ident_b)
            nc.scalar.copy(vh[h][:, jc, :], tp)

    # -------- FSMN (d, t layout), in place on vT; result lands back in vT -----
    for c in range(NC):
        # vm = v * m (in place)
        nc.gpsimd.tensor_tensor(vT[c], vT[c], mrow, op=OP.mult)
        if FSMN_DT == "f32":
            acc = p_main.tile([128, T], F32, name=f"facc{c}", tag="facc")
            # center tap first (w[5]+1 folds the residual), full width
            nc.any.tensor_scalar_mul(acc, vT[c], wadj[:, c, PAD : PAD + 1])
            src_t = vT[c]
        else:
            vmb = p_main.tile([128, T], BF16, name=f"vmb{c}", tag="fscratch")
            nc.any.tensor_copy(vmb, vT[c])
            acc = p_main.tile([128, T], BF16, name=f"facc{c}", tag="facc")
            nc.any.tensor_scalar_mul(acc, vmb, wadj[:, c, PAD : PAD + 1])
            src_t = vmb
        for k in list(range(0, PAD)) + list(range(PAD + 1, KS)):
            s = k - PAD
            lo, hi = max(0, -s), T - max(0, s)
            nc.vector.scalar_tensor_tensor(
                acc[:, lo:hi],
                src_t[:, lo + s : hi + s],
                wadj[:, c, k : k + 1],
                acc[:, lo:hi],
                OP.mult,
                OP.add,
            )
        if FSMN_DT == "f32":
            nc.gpsimd.tensor_tensor(acc, acc, mrow, op=OP.mult)
            # final masked conv + bias overwrites vT[c] (all tap reads are done)
            nc.any.tensor_scalar_add(vT[c], acc, bo[:, c : c + 1])
        else:
            # residual in f32: out = (conv_bf + vm) * m + bo; vm*m = vm
            facc2 = p_main.tile([128, T], F32, name=f"facc2{c}", tag="fscratch")
            nc.vector.tensor_tensor(facc2, acc, mrow, op=OP.mult)
            nc.gpsimd.tensor_tensor(vT[c], facc2, vT[c], op=OP.add)
            nc.any.tensor_scalar_add(vT[c], vT[c], bo[:, c : c + 1])

    # ---------------- attention ----------------
    zd = dram.tile([H * T], F32, name="zd", tag="zd", bufs=1)
    ctxT = [
        p_main.tile([128, T], att_store, name=f"ctxT{h}", tag=f"ctxT{h}")
        for h in range(H)
    ]
    for h in range(H):
        for ih in range(2):  # query halves of 1024
            isl = slice(ih * 1024, (ih + 1) * 1024)
            ctx_ps = ps.tile([128, 1024], F32, name="ctx_ps", tag="actx", bufs=1)
            z_ps = ps.tile([1, 1024], F32, name="z_ps", tag="z", bufs=1)
            for jc in range(TKC):
                for q2 in range(2):
                    s_ps = ps.tile([128, 512], F32, name="s_ps", tag="s", bufs=4)
                    i0 = ih * 1024 + q2 * 512
                    nc.tensor.matmul(
                        s_ps,
                        kTc[h][:, jc * 128 : (jc + 1) * 128],
                        qT[h][:, i0 : i0 + 512],
                        start=True,
                        stop=True,
                        skip_group_check=True,
                    )
                    eT = work.tile([128, 512], att_store, name="eT", tag="eT", bufs=4)
                    nc.scalar.activation(
                        eT, s_ps, AF.Exp, bias=mbias[:, jc : jc + 1], scale=SCALE
                    )
                    qsl = slice(q2 * 512, (q2 + 1) * 512)
                    nc.tensor.matmul(
                        ctx_ps[:, qsl],
                        vh[h][:, jc, :],
                        eT,
                        start=(jc == 0),
                        stop=(jc == TKC - 1),
                        skip_group_check=True,
                    )
                    nc.tensor.matmul(
                        z_ps[:, qsl],
                        ones_att,
                        eT,
                        start=(jc == 0),
                        stop=(jc == TKC - 1),
                        skip_group_check=True,
                    )
            z_sb = work.tile([1, 1024], F32, name="z_sb", tag="z_sb", bufs=1)
            nc.scalar.copy(z_sb, z_ps)
            rz = work.tile([1, 1024], F32, name="rz", tag="rz", bufs=1)
            nc.vector.reciprocal(rz, z_sb)
            zsl = slice(h * T + ih * 1024, h * T + ih * 1024 + 1024)
            nc.sync.dma_start(out=zd[zsl], in_=rz)
            zb = work.tile([128, 1024], BF16, name="zb", tag="zb", bufs=2)
            nc.gpsimd.dma_start(
                out=zb,
                in_=bass.AP(
                    tensor=zd.tensor,
                    offset=zd.offset + h * T + ih * 1024,
                    ap=[[0, 128], [1, 1024]],
                ),
            )
            if NORM == "psum":
                nc.vector.tensor_tensor(ctxT[h][:, isl], ctx_ps, zb, op=OP.mult)
            else:
                # fast psum release: copy unnormalized, then normalize in SBUF
                nc.vector.tensor_copy(ctxT[h][:, isl], ctx_ps)
                nc.vector.tensor_tensor(
                    ctxT[h][:, isl], ctxT[h][:, isl], zb, op=OP.mult
                )

    # ---------------- out projection + fsmn add ----------------
    for tb in range(NT):
        op_ps = ps.tile([128, 512], F32, name="op_ps", tag="s", bufs=4)
        for h in range(H):
            nc.tensor.matmul(
                op_ps,
                ctxT[h][:, tb * 128 : (tb + 1) * 128],
                wo[:, h, :],
                start=(h == 0),
                stop=(h == H - 1),
            )
        # transpose this t-block of fsmn into natural layout
        ftp = ps.tile([128, 512], F32, name="ftp", tag="z", bufs=1)
        for c in range(NC):
            nc.tensor.transpose(
                ftp[:, c * 128 : (c + 1) * 128],
                vT[c][:, tb * 128 : (tb + 1) * 128],
                ident,
            )
        f_sb = work.tile([128, D], F32, name="f_sb", tag="f_sb", bufs=2)
        nc.scalar.copy(f_sb, ftp)
        o_sb = work.tile([128, D], F32, name="o_sb", tag="o_sb", bufs=2)
        nc.vector.tensor_tensor(o_sb, op_ps, f_sb, op=OP.add)
        nc.sync.dma_start(out=out_d[tb * 128 : (tb + 1) * 128, :], in_=o_sb)

    main_cm.__exit__(None, None, None)
    stack.close()


_CACHE = {}


def _build(TK):
    key = (QKV_DT, ATT_DT, REPS, TK, NORM, FSMN_DT)
    if key in _CACHE:
        return _CACHE[key]
    nc = bacc.Bacc(
        "TRN2",
        target_bir_lowering=False,
        debug=False,
        enable_asserts=False,
        num_devices=NCORES,
    )
    aps = (
        nc.dram_tensor("x", (T, D), F32, kind="ExternalInput").ap(),
        nc.dram_tensor("mask", (T,), F32, kind="ExternalInput").ap(),
        nc.dram_tensor("xc", (TK, D), F32, kind="ExternalInput").ap(),
        nc.dram_tensor("cbias", (TK,), F32, kind="ExternalInput").ap(),
        nc.dram_tensor("Wqkv", (D, 3 * D), F32, kind="ExternalInput").ap(),
        nc.dram_tensor("bqkv", (3 * D,), F32, kind="ExternalInput").ap(),
        nc.dram_tensor("Wout", (D, D), F32, kind="ExternalInput").ap(),
        nc.dram_tensor("bout", (D,), F32, kind="ExternalInput").ap(),
        nc.dram_tensor("fsmn_w", (D, 1, KS), F32, kind="ExternalInput").ap(),
        nc.dram_tensor("out", (T, D), F32, kind="ExternalOutput").ap(),
    )
    with tile.TileContext(nc) as tc:
        for rep in range(REPS):
            build_kernel_body(tc, aps, TK, rep)
    nc.compile()
    _CACHE[key] = nc
    return nc


def _compact(x_b, mask_b, TK):
    """Host-side gather of unmasked token rows, padded to TK."""
    idx = np.nonzero(mask_b != 0)[0]
    n = len(idx)
    xc = np.zeros((TK, x_b.shape[1]), np.float32)
    xc[:n] = x_b[idx[:TK]]
    cb = np.full((TK,), MASK_NEG, np.float32)
    cb[:n] = 0.0
    return xc, cb


def kernel(x, mask, Wqkv, bqkv, Wout, bout, fsmn_w):
    x = np.ascontiguousarray(np.asarray(x, dtype=np.float32))
    mask = np.ascontiguousarray(np.asarray(mask, dtype=np.float32))
    Wqkv = np.ascontiguousarray(np.asarray(Wqkv, dtype=np.float32))
    bqkv = np.ascontiguousarray(np.asarray(bqkv, dtype=np.float32))
    Wout = np.ascontiguousarray(np.asarray(Wout, dtype=np.float32))
    bout = np.ascontiguousarray(np.asarray(bout, dtype=np.float32))
    fsmn_w = np.ascontiguousarray(np.asarray(fsmn_w, dtype=np.float32))

    counts = [int((mask[b, 0] != 0).sum()) for b in range(NCORES)]
    TK = min(T, max(256, int(-(-max(counts) // 128) * 128)))

    nc = _build(TK)
    in_maps = []
    for b in range(NCORES):
        xc, cb = _compact(x[b], mask[b, 0], TK)
        in_maps.append(
            {
                "x": x[b],
                "mask": np.ascontiguousarray(mask[b, 0]),
                "xc": xc,
                "cbias": cb,
                "Wqkv": Wqkv,
                "bqkv": bqkv,
                "Wout": Wout,
                "bout": bout,
                "fsmn_w": fsmn_w,
            }
        )
    trace = os.environ.get("SANM_TRACE", "0") == "1"
    if trace:
        try:
            import antenv.axon_hooks  # noqa: F401
        except ImportError:
            trace = False
    res = bass_utils.run_bass_kernel_spmd(
        nc, in_maps, core_ids=list(range(NCORES)), trace=trace
    )
    if trace and res.exec_time_ns is not None:
        print(f"HW exec time: {res.exec_time_ns} ns")
    out = np.stack([res.results[b]["out"] for b in range(NCORES)], axis=0)
    return out


if __name__ == "__main__":
    rng = np.random.default_rng(0)
    ins = {
        "x": rng.standard_normal((NCORES, T, D), dtype=np.float32),
        "mask": rng.integers(0, 2, (NCORES, 1, T)).astype(np.float32),
        "Wqkv": (rng.standard_normal((D, 3 * D)) * 0.02).astype(np.float32),
        "bqkv": np.zeros((3 * D,), np.float32),
        "Wout": (rng.standard_normal((D, D)) * 0.02).astype(np.float32),
        "bout": np.zeros((D,), np.float32),
        "fsmn_w": (rng.standard_normal((D, 1, KS)) * 0.1).astype(np.float32),
    }
    out = kernel(**ins)
    print(out.shape, out.dtype, float(np.abs(out).max()))



# revision 7
# speedup vs baseline: 3208.2600x; 3208.2600x over previous
"""Trainium2 Bass kernel for MultiHeadedAttentionSANM.

Per-core (data-parallel over batch, 8 cores, B=1 each):
  - qkv^T = (x @ Wqkv)^T on PE (float32r): q^T and full v^T (FSMN needs all
    tokens); k^T and a second v^T are computed only for the ~50% of tokens
    with mask=1, gathered host-side into a compact x_c (TK tokens).
  - FSMN: depthwise conv over time in (d, t) layout on DVE (f32), in place
    on v^T with partial-width taps; mask-muls on GPSIMD.
  - attention: scores computed transposed (compact keys on partitions) so the
    exp output feeds the ctx matmul directly as the rhs stream; masked/padded
    keys get a -30000 exp bias. Softmax denominator Z via a ones-weight PE
    pass; normalization is delayed all the way to the out-proj epilogue
    (per-head PSUM + per-partition 1/Z scalars).
"""

import os
import sys

for _p in ("/opt/trn_rl_repo", "/root/.axon_site/_ro/trn_rl_repo"):
    if os.path.isdir(_p) and _p not in sys.path:
        sys.path.append(_p)

from contextlib import ExitStack

import numpy as np

import concourse.bass as bass
import concourse.mybir as mybir
import concourse.tile as tile
from concourse import bacc
from concourse import bass_utils
from concourse.masks import make_identity

T, D, H, DK, KS, PAD = 2048, 512, 4, 128, 11, 5
NCORES = 8
NT = T // 128          # 16 t-blocks of 128
NC = D // 128          # 4 d-chunks of 128
SCALE = float(DK) ** -0.5
MASK_NEG = -30000.0

F32 = mybir.dt.float32
F32R = mybir.dt.float32r
BF16 = mybir.dt.bfloat16
AF = mybir.ActivationFunctionType
OP = mybir.AluOpType

QKV_DT = os.environ.get("SANM_QKV_DT", "f32r")   # f32r | f32
ATT_DT = os.environ.get("SANM_ATT_DT", "bf16")   # bf16 | f32
REPS = int(os.environ.get("SANM_REPS", "1"))     # timing: repeat body in one NEFF
LOOP = int(os.environ.get("SANM_LOOP", "0"))     # timing: hw For_i loop trip count
NORM = os.environ.get("SANM_NORM", "psum")       # psum | inplace
FSMN_DT = os.environ.get("SANM_FSMN_DT", "f32")  # f32 | bf16


def _bcast_vec(ap, nrows):
    """Broadcast a flat [N] DRAM AP across partitions -> [nrows, N]."""
    return bass.AP(tensor=ap.tensor, offset=ap.offset, ap=[[0, nrows]] + list(ap.ap))


def _tiles(total, step=512):
    out, p = [], 0
    while p < total:
        n = min(step, total - p)
        rem = total - p - n
        if 0 < rem < 256:  # avoid <256-wide f32r tails (4 cyc/row penalty)
            n = (n + rem) // 2
            n = (n + 127) // 128 * 128
        out.append((p, n))
        p += n
    return out


def build_kernel_body(tc, aps, TK, rep=0):
    nc = tc.nc
    x_d, mask_d, xc_d, cbias_d, wqkv_d, bqkv_d, wout_d, bout_d, fw_d, out_d = aps
    R = f"r{rep}_" if rep else ""
    TKC = TK // 128  # compact key chunks

    att_store = F32 if ATT_DT == "f32" else BF16
    qkv_store = F32R if QKV_DT == "f32r" else F32

    stack = ExitStack()
    consts = stack.enter_context(tc.tile_pool(name=R + "consts", bufs=1))
    work = stack.enter_context(tc.tile_pool(name=R + "work", bufs=2))
    ps = stack.enter_context(tc.tile_pool(name=R + "ps", bufs=1, space="PSUM"))
    dram = stack.enter_context(tc.tile_pool(name=R + "dram", bufs=2, space="DRAM"))

    # p_main holds all long-lived tensors (whole kernel); p_x nests inside it
    # (LIFO) and is released after the qkv matmuls to reclaim x^T space.
    main_cm = tc.tile_pool(name=R + "p_main", bufs=1)
    x_cm = tc.tile_pool(name=R + "p_x", bufs=1)
    p_main = main_cm.__enter__()
    p_x = x_cm.__enter__()

    # ---------------- constants ----------------
    ident = consts.tile([128, 128], F32, name="ident", tag="ident")
    make_identity(nc, ident)
    ident_b = consts.tile([128, 128], att_store, name="ident_b", tag="ident_b")
    nc.vector.tensor_copy(ident_b, ident)

    ones_att = consts.tile([128, 1], att_store, name="ones_att", tag="ones_att")
    nc.vector.memset(ones_att, 1.0)

    # compact-key exp bias (0 valid / -30000 padded), as columns (128, TKC)
    mbias = consts.tile([128, TKC], F32, name="mbias", tag="mbias")
    nc.sync.dma_start(out=mbias, in_=cbias_d.rearrange("(c p) -> p c", p=128))

    # mask broadcast across partitions (128, T) bf16 (exact for 0/1), for FSMN
    mrow = consts.tile([128, T], BF16, name="mrow", tag="mrow")
    nc.gpsimd.dma_start(out=mrow, in_=_bcast_vec(mask_d, 128))

    # biases as per-partition columns
    bq = consts.tile([128, 12], F32, name="bq", tag="bq")
    nc.sync.dma_start(out=bq, in_=bqkv_d.rearrange("(c p) -> p c", p=128))
    bo = consts.tile([128, NC], F32, name="bo", tag="bo")
    nc.sync.dma_start(out=bo, in_=bout_d.rearrange("(c p) -> p c", p=128))

    # fsmn weights (128, NC, KS); center tap += 1 (folds the residual)
    wadj = consts.tile([128, NC, KS], F32, name="wadj", tag="wadj")
    nc.scalar.dma_start(out=wadj, in_=fw_d.rearrange("(c p) o k -> p c (o k)", p=128))
    if FSMN_DT == "f32":
        nc.vector.tensor_scalar_add(
            wadj[:, :, PAD : PAD + 1], wadj[:, :, PAD : PAD + 1], 1.0
        )

    # Wout (128, NC, 512) in attention dtype (staged through a work tile)
    wo = consts.tile([128, NC, D], att_store, name="wo", tag="wo")
    for c in range(NC):
        wos = work.tile([128, D], F32, name="wos", tag="wos", bufs=2)
        nc.scalar.dma_start(out=wos, in_=wout_d[c * 128 : (c + 1) * 128, :])
        nc.vector.tensor_copy(wo[:, c, :], wos)

    # ---------------- x^T and xc^T (PE transposes) ----------------
    xT = [p_x.tile([128, T], qkv_store, name=f"xT{c}", tag=f"xT{c}") for c in range(NC)]
    for ti in range(NT):
        xn = work.tile([128, D], F32, name="xnat", tag="xnat", bufs=3)
        (nc.sync if ti % 2 == 0 else nc.scalar).dma_start(
            out=xn, in_=x_d[ti * 128 : (ti + 1) * 128, :]
        )
        for c in range(NC):
            tp = ps.tile([128, 128], F32, name="tp", tag="s", bufs=4)
            nc.tensor.transpose(tp, xn[:, c * 128 : (c + 1) * 128], ident)
            dst = xT[c][:, ti * 128 : (ti + 1) * 128]
            if (ti + c) % 2 == 0:
                nc.vector.tensor_copy(dst, tp)
            else:
                nc.scalar.copy(dst, tp)
    xcT = [
        p_x.tile([128, TK], qkv_store, name=f"xcT{c}", tag=f"xcT{c}")
        for c in range(NC)
    ]
    for ti in range(TKC):
        xn = work.tile([128, D], F32, name="xnat", tag="xnat", bufs=3)
        nc.scalar.dma_start(out=xn, in_=xc_d[ti * 128 : (ti + 1) * 128, :])
        for c in range(NC):
            tp = ps.tile([128, 128], F32, name="tpc", tag="s", bufs=4)
            nc.tensor.transpose(tp, xn[:, c * 128 : (c + 1) * 128], ident)
            dst = xcT[c][:, ti * 128 : (ti + 1) * 128]
            if (ti + c) % 2 == 0:
                nc.vector.tensor_copy(dst, tp)
            else:
                nc.scalar.copy(dst, tp)

    # ---------------- qkv^T = (x @ Wqkv)^T ----------------
    # q on full tokens; k only compact; v full (FSMN) and compact (attention)
    qT = [p_main.tile([128, T], att_store, name=f"qT{h}", tag=f"qT{h}") for h in range(H)]
    kTc = [p_main.tile([128, TK], att_store, name=f"kTc{h}", tag=f"kTc{h}") for h in range(H)]
    vT = [p_main.tile([128, T], F32, name=f"vT{c}", tag=f"vT{c}") for c in range(NC)]
    vcT = [p_main.tile([128, TK], att_store, name=f"vcT{c}", tag=f"vcT{c}") for c in range(NC)]

    def project(f, src, tspans, sink):
        """psum[128, n] = Wqkv[:, f-block].T @ src over d-chunks, then sink."""
        wqf = work.tile([128, NC, 128], qkv_store, name="wqf", tag="wqf", bufs=3)
        wqf_src = wqkv_d[:, f * 128 : (f + 1) * 128].rearrange("(c p) f -> p c f", p=128)
        if QKV_DT == "f32r":
            wqf_src = wqf_src.bitcast(F32R)
        (nc.scalar if f % 2 else nc.sync).dma_start(out=wqf, in_=wqf_src)
        for t0, n in tspans:
            mm = ps.tile([128, 512], F32, name="mmq", tag="s", bufs=4)
            for dc in range(NC):
                nc.tensor.matmul(
                    mm[:, :n],
                    wqf[:, dc, :],
                    src[dc][:, t0 : t0 + n],
                    start=(dc == 0),
                    stop=(dc == NC - 1),
                )
            sink(mm, t0, n)

    for f in range(4):  # q: full tokens -> qT (bf16), bias via ACT
        def sink_q(mm, t0, n, f=f):
            nc.scalar.activation(
                qT[f][:, t0 : t0 + n], mm[:, :n], AF.Identity,
                bias=bq[:, f : f + 1], scale=1.0,
            )
        project(f, xT, _tiles(T), sink_q)
    for f in range(4, 8):  # k: compact tokens -> kTc
        def sink_k(mm, t0, n, f=f):
            nc.scalar.activation(
                kTc[f - 4][:, t0 : t0 + n], mm[:, :n], AF.Identity,
                bias=bq[:, f : f + 1], scale=1.0,
            )
        project(f, xcT, _tiles(TK), sink_k)
    for f in range(8, 12):  # v full tokens (FSMN), f32, bias via DVE
        def sink_v(mm, t0, n, f=f):
            nc.vector.tensor_scalar_add(
                vT[f - 8][:, t0 : t0 + n], mm[:, :n], bq[:, f : f + 1]
            )
        project(f, xT, _tiles(T), sink_v)
    for f in range(8, 12):  # v compact tokens (attention), att dtype
        def sink_vc(mm, t0, n, f=f):
            nc.scalar.activation(
                vcT[f - 8][:, t0 : t0 + n], mm[:, :n], AF.Identity,
                bias=bq[:, f : f + 1], scale=1.0,
            )
        project(f, xcT, _tiles(TK), sink_vc)
    x_cm.__exit__(None, None, None)  # frees xT, xcT

    # ------------- compact v natural (PE transposes of vcT, att dtype) --------
    vh = [
        p_main.tile([128, TKC, 128], att_store, name=f"vh{h}", tag=f"vh{h}")
        for h in range(H)
    ]
    for h in range(H):
        for jc in range(TKC):
            tp = ps.tile([128, 128], att_store, name="tpv", tag="s", bufs=4)
            nc.tensor.transpose(tp, vcT[h][:, jc * 128 : (jc + 1) * 128], ident_b)
            nc.scalar.copy(vh[h][:, jc, :], tp)

    # -------- FSMN (d, t layout), in place on vT; result lands back in vT -----
    for c in range(NC):
        # vm = v * m (in place)
        nc.gpsimd.tensor_tensor(vT[c], vT[c], mrow, op=OP.mult)
        if FSMN_DT == "f32":
            acc = p_main.tile([128, T], F32, name=f"facc{c}", tag="facc")
            # center tap first (w[5]+1 folds the residual), full width
            nc.any.tensor_scalar_mul(acc, vT[c], wadj[:, c, PAD : PAD + 1])
            src_t = vT[c]
        else:
            vmb = p_main.tile([128, T], BF16, name=f"vmb{c}", tag="fscratch")
            nc.any.tensor_copy(vmb, vT[c])
            acc = p_main.tile([128, T], BF16, name=f"facc{c}", tag="facc")
            nc.any.tensor_scalar_mul(acc, vmb, wadj[:, c, PAD : PAD + 1])
            src_t = vmb
        for k in list(range(0, PAD)) + list(range(PAD + 1, KS)):
            s = k - PAD
            lo, hi = max(0, -s), T - max(0, s)
            nc.vector.scalar_tensor_tensor(
                acc[:, lo:hi],
                src_t[:, lo + s : hi + s],
                wadj[:, c, k : k + 1],
                acc[:, lo:hi],
                OP.mult,
                OP.add,
            )
        if FSMN_DT == "f32":
            nc.gpsimd.tensor_tensor(acc, acc, mrow, op=OP.mult)
            # final masked conv + bias overwrites vT[c] (all tap reads are done)
            nc.any.tensor_scalar_add(vT[c], acc, bo[:, c : c + 1])
        else:
            # residual in f32: out = (conv_bf + vm) * m + bo; vm*m = vm
            facc2 = p_main.tile([128, T], F32, name=f"facc2{c}", tag="fscratch")
            nc.vector.tensor_tensor(facc2, acc, mrow, op=OP.mult)
            nc.gpsimd.tensor_tensor(vT[c], facc2, vT[c], op=OP.add)
            nc.any.tensor_scalar_add(vT[c], vT[c], bo[:, c : c + 1])

    # ---------------- attention ----------------
    zd = dram.tile([H * T], F32, name="zd", tag="zd", bufs=1)
    ctxT = [
        p_main.tile([128, T], att_store, name=f"ctxT{h}", tag=f"ctxT{h}")
        for h in range(H)
    ]
    for h in range(H):
        for ih in range(2):  # query halves of 1024
            isl = slice(ih * 1024, (ih + 1) * 1024)
            ctx_ps = ps.tile([128, 1024], F32, name="ctx_ps", tag="actx", bufs=1)
            z_ps = ps.tile([1, 1024], F32, name="z_ps", tag="z", bufs=1)
            for jc in range(TKC):
                for q2 in range(2):
                    s_ps = ps.tile([128, 512], F32, name="s_ps", tag="s", bufs=4)
                    i0 = ih * 1024 + q2 * 512
                    nc.tensor.matmul(
                        s_ps,
                        kTc[h][:, jc * 128 : (jc + 1) * 128],
                        qT[h][:, i0 : i0 + 512],
                        start=True,
                        stop=True,
                        skip_group_check=True,
                    )
                    eT = work.tile([128, 512], att_store, name="eT", tag="eT", bufs=4)
                    nc.scalar.activation(
                        eT, s_ps, AF.Exp, bias=mbias[:, jc : jc + 1], scale=SCALE
                    )
                    qsl = slice(q2 * 512, (q2 + 1) * 512)
                    nc.tensor.matmul(
                        ctx_ps[:, qsl],
                        vh[h][:, jc, :],
                        eT,
                        start=(jc == 0),
                        stop=(jc == TKC - 1),
                        skip_group_check=True,
                    )
                    nc.tensor.matmul(
                        z_ps[:, qsl],
                        ones_att,
                        eT,
                        start=(jc == 0),
                        stop=(jc == TKC - 1),
                        skip_group_check=True,
                    )
            z_sb = work.tile([1, 1024], F32, name="z_sb", tag="z_sb", bufs=1)
            nc.scalar.copy(z_sb, z_ps)
            rz = work.tile([1, 1024], F32, name="rz", tag="rz", bufs=1)
            nc.vector.reciprocal(rz, z_sb)
            zsl = slice(h * T + ih * 1024, h * T + ih * 1024 + 1024)
            nc.sync.dma_start(out=zd[zsl], in_=rz)
            zb = work.tile([128, 1024], BF16, name="zb", tag="zb", bufs=2)
            nc.gpsimd.dma_start(
                out=zb,
                in_=bass.AP(
                    tensor=zd.tensor,
                    offset=zd.offset + h * T + ih * 1024,
                    ap=[[0, 128], [1, 1024]],
                ),
            )
            if NORM == "psum":
                nc.vector.tensor_tensor(ctxT[h][:, isl], ctx_ps, zb, op=OP.mult)
            else:
                # fast psum release: copy unnormalized, then normalize in SBUF
                nc.vector.tensor_copy(ctxT[h][:, isl], ctx_ps)
                nc.vector.tensor_tensor(
                    ctxT[h][:, isl], ctxT[h][:, isl], zb, op=OP.mult
                )

    # ---------------- out projection + fsmn add ----------------
    for tb in range(NT):
        op_ps = ps.tile([128, 512], F32, name="op_ps", tag="s", bufs=4)
        for h in range(H):
            nc.tensor.matmul(
                op_ps,
                ctxT[h][:, tb * 128 : (tb + 1) * 128],
                wo[:, h, :],
                start=(h == 0),
                stop=(h == H - 1),
            )
        # transpose this t-block of fsmn into natural layout
        ftp = ps.tile([128, 512], F32, name="ftp", tag="z", bufs=1)
        for c in range(NC):
            nc.tensor.transpose(
                ftp[:, c * 128 : (c + 1) * 128],
                vT[c][:, tb * 128 : (tb + 1) * 128],
                ident,
            )
        f_sb = work.tile([128, D], F32, name="f_sb", tag="f_sb", bufs=2)
        nc.scalar.copy(f_sb, ftp)
        o_sb = work.tile([128, D], F32, name="o_sb", tag="o_sb", bufs=2)
        nc.vector.tensor_tensor(o_sb, op_ps, f_sb, op=OP.add)
        nc.sync.dma_start(out=out_d[tb * 128 : (tb + 1) * 128, :], in_=o_sb)

    main_cm.__exit__(None, None, None)
    stack.close()


_CACHE = {}
_FN_CACHE = {}


def make_sharded_fn(nc, n_cores=NCORES):
    """Build a reusable jitted executable for `nc` (done once per build).

    run_bass_kernel_spmd creates a fresh jax.jit per call, so every
    invocation re-traces, re-lowers and re-loads the NEFF; caching the
    jitted callable makes repeat kernel() calls cost only transfer+exec.
    """
    import jax
    from jax.experimental.shard_map import shard_map
    from jax.sharding import Mesh, PartitionSpec

    from concourse import bass2jax
    from concourse.bass2jax import _bass_exec_p, install_neuronx_cc_hook

    install_neuronx_cc_hook()
    partition_name = nc.partition_id_tensor.name if nc.partition_id_tensor else None
    in_names, out_names, out_avals, zero_outs = [], [], [], []
    for alloc in nc.m.functions[0].allocations:
        if not isinstance(alloc, mybir.MemoryLocationSet):
            continue
        name = alloc.memorylocations[0].name
        if alloc.kind == "ExternalInput":
            if name != partition_name:
                in_names.append(name)
        elif alloc.kind == "ExternalOutput":
            out_names.append(name)
            shape = tuple(alloc.tensor_shape)
            dtype = mybir.dt.np(alloc.dtype)
            out_avals.append(jax.core.ShapedArray(shape, dtype))
            zero_outs.append(np.zeros(shape, dtype))
    n_params = len(in_names)
    all_in_names = list(in_names) + list(out_names)
    if partition_name is not None:
        all_in_names.append(partition_name)

    def _body(*args):
        operands = list(args)
        if partition_name is not None:
            operands.append(bass2jax.partition_id_tensor())
        outs = _bass_exec_p.bind(
            *operands,
            out_avals=tuple(out_avals),
            in_names=tuple(all_in_names),
            out_names=tuple(out_names),
            lowering_input_output_aliases=(),
            sim_require_finite=True,
            sim_require_nnan=True,
            nc=nc,
        )
        return tuple(outs)

    devices = jax.devices()[:n_cores]
    mesh = Mesh(np.asarray(devices), ("core",))
    n_outs = len(out_avals)
    in_specs = (PartitionSpec("core"),) * (n_params + n_outs)
    out_specs = (PartitionSpec("core"),) * n_outs
    fn = jax.jit(
        shard_map(
            _body, mesh=mesh, in_specs=in_specs, out_specs=out_specs, check_rep=False
        ),
        keep_unused=True,
    )
    return fn, in_names, out_names, zero_outs


def run_cached(nc, in_maps, key):
    """Execute via a cached jitted executable (falls back to the slow path)."""
    import jax

    if key not in _FN_CACHE:
        _FN_CACHE[key] = make_sharded_fn(nc)
    fn, in_names, out_names, zero_outs = _FN_CACHE[key]
    n = len(in_maps)
    concat_in = [
        np.concatenate([np.asarray(in_maps[c][name]) for c in range(n)], axis=0)
        for name in in_names
    ]
    concat_zeros = [
        np.zeros((n * z.shape[0], *z.shape[1:]), z.dtype) for z in zero_outs
    ]
    out_arrs = fn(*concat_in, *concat_zeros)
    outs = [np.asarray(a) for a in out_arrs]
    return [
        {
            name: outs[i].reshape(n, outs[i].shape[0] // n, *outs[i].shape[1:])[c]
            for i, name in enumerate(out_names)
        }
        for c in range(n)
    ]


def _build(TK):
    key = (QKV_DT, ATT_DT, REPS, TK, NORM, FSMN_DT, LOOP)
    if key in _CACHE:
        return _CACHE[key]
    nc = bacc.Bacc(
        "TRN2",
        target_bir_lowering=False,
        debug=False,
        enable_asserts=False,
        num_devices=NCORES,
    )
    aps = (
        nc.dram_tensor("x", (T, D), F32, kind="ExternalInput").ap(),
        nc.dram_tensor("mask", (T,), F32, kind="ExternalInput").ap(),
        nc.dram_tensor("xc", (TK, D), F32, kind="ExternalInput").ap(),
        nc.dram_tensor("cbias", (TK,), F32, kind="ExternalInput").ap(),
        nc.dram_tensor("Wqkv", (D, 3 * D), F32, kind="ExternalInput").ap(),
        nc.dram_tensor("bqkv", (3 * D,), F32, kind="ExternalInput").ap(),
        nc.dram_tensor("Wout", (D, D), F32, kind="ExternalInput").ap(),
        nc.dram_tensor("bout", (D,), F32, kind="ExternalInput").ap(),
        nc.dram_tensor("fsmn_w", (D, 1, KS), F32, kind="ExternalInput").ap(),
        nc.dram_tensor("out", (T, D), F32, kind="ExternalOutput").ap(),
    )
    with tile.TileContext(nc) as tc:
        if LOOP > 0:
            # hw loop: NEFF size is constant in trip count, so a large trip
            # count isolates per-rep device time from dispatch overhead
            with tc.For_i(0, LOOP, 1):
                build_kernel_body(tc, aps, TK, 0)
        else:
            for rep in range(REPS):
                build_kernel_body(tc, aps, TK, rep)
    nc.compile()
    _CACHE[key] = nc
    return nc


def _compact(x_b, mask_b, TK):
    """Host-side gather of unmasked token rows, padded to TK."""
    idx = np.nonzero(mask_b != 0)[0]
    n = len(idx)
    xc = np.zeros((TK, x_b.shape[1]), np.float32)
    xc[:n] = x_b[idx[:TK]]
    cb = np.full((TK,), MASK_NEG, np.float32)
    cb[:n] = 0.0
    return xc, cb


def kernel(x, mask, Wqkv, bqkv, Wout, bout, fsmn_w):
    x = np.ascontiguousarray(np.asarray(x, dtype=np.float32))
    mask = np.ascontiguousarray(np.asarray(mask, dtype=np.float32))
    Wqkv = np.ascontiguousarray(np.asarray(Wqkv, dtype=np.float32))
    bqkv = np.ascontiguousarray(np.asarray(bqkv, dtype=np.float32))
    Wout = np.ascontiguousarray(np.asarray(Wout, dtype=np.float32))
    bout = np.ascontiguousarray(np.asarray(bout, dtype=np.float32))
    fsmn_w = np.ascontiguousarray(np.asarray(fsmn_w, dtype=np.float32))

    counts = [int((mask[b, 0] != 0).sum()) for b in range(NCORES)]
    TK = min(T, max(256, int(-(-max(counts) // 128) * 128)))

    nc = _build(TK)
    in_maps = []
    for b in range(NCORES):
        xc, cb = _compact(x[b], mask[b, 0], TK)
        in_maps.append(
            {
                "x": x[b],
                "mask": np.ascontiguousarray(mask[b, 0]),
                "xc": xc,
                "cbias": cb,
                "Wqkv": Wqkv,
                "bqkv": bqkv,
                "Wout": Wout,
                "bout": bout,
                "fsmn_w": fsmn_w,
            }
        )
    try:
        results = run_cached(nc, in_maps, key=(id(nc), TK))
    except Exception:
        res = bass_utils.run_bass_kernel_spmd(
            nc, in_maps, core_ids=list(range(NCORES)), trace=False
        )
        results = res.results
    out = np.stack([results[b]["out"] for b in range(NCORES)], axis=0)
    return out


if __name__ == "__main__":
    rng = np.random.default_rng(0)
    ins = {
        "x": rng.standard_normal((NCORES, T, D), dtype=np.float32),
        "mask": rng.integers(0, 2, (NCORES, 1, T)).astype(np.float32),
        "Wqkv": (rng.standard_normal((D, 3 * D)) * 0.02).astype(np.float32),
        "bqkv": np.zeros((3 * D,), np.float32),
        "Wout": (rng.standard_normal((D, D)) * 0.02).astype(np.float32),
        "bout": np.zeros((D,), np.float32),
        "fsmn_w": (rng.standard_normal((D, 1, KS)) * 0.1).astype(np.float32),
    }
    out = kernel(**ins)
    print(out.shape, out.dtype, float(np.abs(out).max()))



# revision 29
# speedup vs baseline: 3701.0734x; 1.1536x over previous
"""Trainium2 Bass kernel for MultiHeadedAttentionSANM.

Per-core (data-parallel over batch, 8 cores, B=1 each):
  - qkv^T = (x @ Wqkv)^T on PE (float32r): q^T and full v^T (FSMN needs all
    tokens); k^T and a second v^T are computed only for the ~50% of tokens
    with mask=1, gathered host-side into a compact x_c (TK tokens).
  - FSMN: depthwise conv over time in (d, t) layout on DVE (f32), in place
    on v^T with partial-width taps; mask-muls on GPSIMD.
  - attention: scores computed transposed (compact keys on partitions) so the
    exp output feeds the ctx matmul directly as the rhs stream; masked/padded
    keys get a -30000 exp bias. Softmax denominator Z via a ones-weight PE
    pass; normalization is delayed all the way to the out-proj epilogue
    (per-head PSUM + per-partition 1/Z scalars).
"""

import os
import sys

for _p in ("/opt/trn_rl_repo", "/root/.axon_site/_ro/trn_rl_repo"):
    if os.path.isdir(_p) and _p not in sys.path:
        sys.path.append(_p)

from contextlib import ExitStack

import numpy as np

import concourse.bass as bass
import concourse.mybir as mybir
import concourse.tile as tile
from concourse import bacc
from concourse import bass_utils
from concourse.masks import make_identity

T, D, H, DK, KS, PAD = 2048, 512, 4, 128, 11, 5
NCORES = 8
NT = T // 128          # 16 t-blocks of 128
NC = D // 128          # 4 d-chunks of 128
SCALE = float(DK) ** -0.5
MASK_NEG = -30000.0

F32 = mybir.dt.float32
F32R = mybir.dt.float32r
BF16 = mybir.dt.bfloat16
F16 = mybir.dt.float16
AF = mybir.ActivationFunctionType
OP = mybir.AluOpType

QKV_DT = os.environ.get("SANM_QKV_DT", "f32r")   # f32r | f32
ATT_DT = os.environ.get("SANM_ATT_DT", "bf16")   # bf16 | f32
REPS = int(os.environ.get("SANM_REPS", "1"))     # timing: repeat body in one NEFF
LOOP = int(os.environ.get("SANM_LOOP", "0"))     # timing: hw For_i loop trip count
NORM = os.environ.get("SANM_NORM", "psum")       # psum | inplace
FSMN_DT = os.environ.get("SANM_FSMN_DT", "f32")  # f32 | bf16


def _bcast_vec(ap, nrows):
    """Broadcast a flat [N] DRAM AP across partitions -> [nrows, N]."""
    return bass.AP(tensor=ap.tensor, offset=ap.offset, ap=[[0, nrows]] + list(ap.ap))


def _tiles(total, step=512):
    out, p = [], 0
    while p < total:
        n = min(step, total - p)
        rem = total - p - n
        if 0 < rem < 256:  # avoid <256-wide f32r tails (4 cyc/row penalty)
            n = (n + rem) // 2
            n = (n + 127) // 128 * 128
        out.append((p, n))
        p += n
    return out


def build_kernel_body(tc, aps, TK, rep=0):
    nc = tc.nc
    x_d, mask_d, xc_d, cbias_d, wqkv_d, bqkv_d, wout_d, bout_d, fw_d, out_d = aps
    R = f"r{rep}_" if rep else ""
    TKC = TK // 128  # compact key chunks

    stack = ExitStack()
    consts = stack.enter_context(tc.tile_pool(name=R + "consts", bufs=1))
    work = stack.enter_context(tc.tile_pool(name=R + "work", bufs=2))
    ps = stack.enter_context(tc.tile_pool(name=R + "ps", bufs=1, space="PSUM"))
    dram = stack.enter_context(tc.tile_pool(name=R + "dram", bufs=2, space="DRAM"))

    # p_main holds all long-lived tensors (whole kernel); p_x nests inside it
    # (LIFO) and is released after the qkv matmuls to reclaim x^T space.
    main_cm = tc.tile_pool(name=R + "p_main", bufs=1)
    x_cm = tc.tile_pool(name=R + "p_x", bufs=1)
    p_main = main_cm.__enter__()
    p_x = x_cm.__enter__()

    # ---------------- constants ----------------
    ident = consts.tile([128, 128], F32, name="ident", tag="ident")
    make_identity(nc, ident)
    ident_r = consts.tile([128, 128], F32R, name="ident_r", tag="ident_r")
    nc.vector.tensor_copy(ident_r, ident.bitcast(F32R))
    ident_b = consts.tile([128, 128], BF16, name="ident_b", tag="ident_b")
    nc.vector.tensor_copy(ident_b, ident)
    ident_h = consts.tile([128, 128], F16, name="ident_h", tag="ident_h")
    nc.vector.tensor_copy(ident_h, ident)

    ones_att = consts.tile([128, 1], BF16, name="ones_att", tag="ones_att")
    nc.vector.memset(ones_att, 1.0)

    # compact-key exp bias (0 valid / -30000 padded), as columns (128, TKC)
    mbias = consts.tile([128, TKC], F32, name="mbias", tag="mbias")
    nc.sync.dma_start(out=mbias, in_=cbias_d.rearrange("(c p) -> p c", p=128))

    # mask broadcast across partitions (128, T) bf16 (exact for 0/1), for FSMN
    mrow = consts.tile([128, T], BF16, name="mrow", tag="mrow")
    nc.gpsimd.dma_start(out=mrow, in_=_bcast_vec(mask_d, 128))

    # biases as per-partition columns
    bq = consts.tile([128, 12], F32, name="bq", tag="bq")
    nc.sync.dma_start(out=bq, in_=bqkv_d.rearrange("(c p) -> p c", p=128))
    bo = consts.tile([128, NC], F32, name="bo", tag="bo")
    nc.sync.dma_start(out=bo, in_=bout_d.rearrange("(c p) -> p c", p=128))

    # fsmn weights (128, NC, KS); center tap += 1 (folds the residual)
    wadj = consts.tile([128, NC, KS], F32, name="wadj", tag="wadj")
    nc.scalar.dma_start(out=wadj, in_=fw_d.rearrange("(c p) o k -> p c (o k)", p=128))
    nc.vector.tensor_scalar_add(
        wadj[:, :, PAD : PAD + 1], wadj[:, :, PAD : PAD + 1], 1.0
    )

    # Wout (128, NC, 512) bf16, loaded directly (host pre-converts to bf16)
    wo = consts.tile([128, NC, D], BF16, name="wo", tag="wo")
    nc.scalar.dma_start(out=wo, in_=wout_d.rearrange("(c p) d -> p c d", p=128))

    # ---------------- x^T and xc^T (PE transposes, bf16 = 1 cyc/row) ---------
    # x/xc arrive bf16 (host pre-converts); single tiles [128, NC, T*] so each
    # 128-token block needs one batched PSUM->SBUF copy instead of four
    xT = p_x.tile([128, NC, T], BF16, name="xT", tag="xT")
    xcT = p_x.tile([128, NC, TK], BF16, name="xcT", tag="xcT")
    for ti in range(NT + TKC):
        if ti < NT:
            src_d, dstT, t0 = x_d, xT, ti * 128
        else:
            src_d, dstT, t0 = xc_d, xcT, (ti - NT) * 128
        xn = work.tile([128, D], BF16, name="xnat", tag="xnat", bufs=3)
        (nc.sync if ti % 2 == 0 else nc.scalar).dma_start(
            out=xn, in_=src_d[t0 : t0 + 128, :]
        )
        tp = ps.tile([128, 512], BF16, name="tp", tag="s", bufs=4)
        for c in range(NC):
            nc.tensor.transpose(
                tp[:, c * 128 : (c + 1) * 128], xn[:, c * 128 : (c + 1) * 128], ident_b
            )
        dst = dstT[:, :, t0 : t0 + 128]
        if ti % 2 == 0:
            nc.vector.tensor_copy(dst, tp)
        else:
            nc.scalar.copy(dst, tp)

    # ---------------- qkv^T = (x @ Wqkv)^T ----------------
    # q on full tokens; k only compact; v full (FSMN) and compact (attention)
    qT = [p_main.tile([128, T], BF16, name=f"qT{h}", tag=f"qT{h}") for h in range(H)]
    kTc = [p_main.tile([128, TK], BF16, name=f"kTc{h}", tag=f"kTc{h}") for h in range(H)]
    # fp16 for the FSMN path: same 2-byte DVE speed, 8x finer mantissa (the
    # conv accumulates at the residual's scale, where bf16 rounding is ~1e-2)
    vT = [p_main.tile([128, T], F16, name=f"vT{c}", tag=f"vT{c}") for c in range(NC)]
    vcT = [p_main.tile([128, TK], BF16, name=f"vcT{c}", tag=f"vcT{c}") for c in range(NC)]

    def project(f, srcT, tspans, sink):
        """psum[128, n] = Wqkv[:, f-block].T @ src over d-chunks, then sink."""
        wqf = work.tile([128, NC, 128], BF16, name="wqf", tag="wqf", bufs=3)
        wqf_src = wqkv_d[:, f * 128 : (f + 1) * 128].rearrange(
            "(c p) f -> p c f", p=128
        )
        (nc.scalar if f % 2 else nc.sync).dma_start(out=wqf, in_=wqf_src)
        for t0, n in tspans:
            mm = ps.tile([128, 512], F32, name="mmq", tag="s", bufs=4)
            for dc in range(NC):
                nc.tensor.matmul(
                    mm[:, :n],
                    wqf[:, dc, :],
                    srcT[:, dc, t0 : t0 + n],
                    start=(dc == 0),
                    stop=(dc == NC - 1),
                )
            sink(mm, t0, n)

    def act_sink(dst, f):
        def sink(mm, t0, n):
            nc.scalar.activation(
                dst[:, t0 : t0 + n], mm[:, :n], AF.Identity,
                bias=bq[:, f : f + 1], scale=1.0,
            )
        return sink

    for f in range(4):  # q: full tokens -> qT (bf16), bias via ACT
        project(f, xT, _tiles(T), act_sink(qT[f], f))
    for f in range(4, 8):  # k: compact tokens -> kTc
        project(f, xcT, _tiles(TK), act_sink(kTc[f - 4], f))
    for f in range(8, 12):  # v compact tokens (attention)
        project(f, xcT, _tiles(TK), act_sink(vcT[f - 8], f))
    for f in range(8, 12):  # v full tokens (FSMN), bf16
        project(f, xT, _tiles(T), act_sink(vT[f - 8], f))
    x_cm.__exit__(None, None, None)  # frees xT, xcT

    # ------------- compact v natural (PE transposes of vcT, batched) ---------
    vh = [
        p_main.tile([128, TKC, 128], BF16, name=f"vh{h}", tag=f"vh{h}")
        for h in range(H)
    ]
    for h in range(H):
        for j0 in range(0, TKC, 4):
            jn = min(4, TKC - j0)
            tp = ps.tile([128, 512], BF16, name="tpv", tag="s", bufs=4)
            for j in range(jn):
                nc.tensor.transpose(
                    tp[:, j * 128 : (j + 1) * 128],
                    vcT[h][:, (j0 + j) * 128 : (j0 + j + 1) * 128],
                    ident_b,
                )
            nc.scalar.copy(vh[h][:, j0 : j0 + jn, :], tp[:, : jn * 128])

    # -------- FSMN (d, t layout), bf16, all on DVE; result lands in vT -------
    # issued chunk-by-chunk interleaved into the attention program order so
    # the DVE queue alternates fsmn taps with attention recip/normalize ops
    vmt = [p_main.tile([128, T], F16, name=f"vmt{c}", tag=f"vmt{c}") for c in range(NC)]
    fac = [p_main.tile([128, T], F16, name=f"fac{c}", tag=f"fac{c}") for c in range(NC)]

    def fsmn_chunk(c):
        # STT (mult+add) has no fast DVE ucode mode, so each tap is a
        # 4x-mode tensor_scalar mult into a scratch plus a 2x-mode add.
        # Accumulate the small off-center taps first and the big residual
        # term (center tap, w+1) last, so rounding happens at tap scale.
        vm, acc = vmt[c], fac[c]
        nc.vector.tensor_tensor(vm, vT[c], mrow, op=OP.mult)
        # center tap first: the only tap covering the full width (edges of
        # shifted taps contribute zero-padding), carries the +1 residual
        nc.vector.tensor_scalar_mul(acc, vm, wadj[:, c, PAD : PAD + 1])
        for kk in list(range(0, PAD)) + list(range(PAD + 1, KS)):
            s = kk - PAD
            lo, hi = max(0, -s), T - max(0, s)
            tap = work.tile([128, T], F16, name="tap", tag="tap", bufs=2)
            nc.vector.tensor_scalar_mul(
                tap[:, lo:hi], vm[:, lo + s : hi + s], wadj[:, c, kk : kk + 1]
            )
            nc.vector.tensor_tensor(
                acc[:, lo:hi], acc[:, lo:hi], tap[:, lo:hi], op=OP.add
            )
        # out = (conv + vm) * m + bo  (center tap carries the +1 residual)
        nc.vector.tensor_tensor(vT[c], acc, mrow, op=OP.mult)
        nc.vector.tensor_scalar_add(vT[c], vT[c], bo[:, c : c + 1])

    # ---------------- attention ----------------
    # per (h, query-block of 512): scores transposed (compact keys on
    # partitions); exp with -30000 pad bias; ctx accumulates over key chunks
    # in one PSUM bank; Z = sum_k exp via esum (Pool adds) + one ones-matmul;
    # 1/Z broadcast across partitions with an SBUF->SBUF DMA; normalize on DVE
    ctxT = [
        p_main.tile([128, T], BF16, name=f"ctxT{h}", tag=f"ctxT{h}")
        for h in range(H)
    ]
    zd = dram.tile([16 * 512], BF16, name="zd", tag="zd", bufs=1)
    it = 0
    for h in range(H):
        for qb in range(4):  # query blocks of 512
            i0 = qb * 512
            if it < NC:
                fsmn_chunk(it)
            ctx_ps = ps.tile([128, 512], F32, name="ctx_ps", tag="actx", bufs=2)
            z_ps = ps.tile([1, 512], F32, name="z_ps", tag="z", bufs=2)
            for jc in range(TKC):
                s_ps = ps.tile([128, 512], F32, name="s_ps", tag="s", bufs=4)
                nc.tensor.matmul(
                    s_ps,
                    kTc[h][:, jc * 128 : (jc + 1) * 128],
                    qT[h][:, i0 : i0 + 512],
                    start=True,
                    stop=True,
                    skip_group_check=True,
                )
                eT = work.tile([128, 512], BF16, name="eT", tag="eT", bufs=4)
                nc.scalar.activation(
                    eT, s_ps, AF.Exp, bias=mbias[:, jc : jc + 1], scale=SCALE
                )
                nc.tensor.matmul(
                    ctx_ps,
                    vh[h][:, jc, :],
                    eT,
                    start=(jc == 0),
                    stop=(jc == TKC - 1),
                    skip_group_check=True,
                )
                # Z accumulates the exact same bf16 exp values in f32 PSUM, so
                # eT rounding cancels between numerator and denominator
                nc.tensor.matmul(
                    z_ps,
                    ones_att,
                    eT,
                    start=(jc == 0),
                    stop=(jc == TKC - 1),
                    skip_group_check=True,
                )
            rz = work.tile([1, 512], BF16, name="rz", tag="rz", bufs=2)
            with nc.allow_low_precision(reason="1/Z applied to bf16 attn weights"):
                nc.vector.reciprocal(rz, z_ps)
            nc.sync.dma_start(out=zd[it * 512 : (it + 1) * 512], in_=rz)
            rzb = work.tile([128, 512], BF16, name="rzb", tag="rzb", bufs=2)
            nc.gpsimd.dma_start(
                out=rzb,
                in_=bass.AP(
                    tensor=zd.tensor,
                    offset=zd.offset + it * 512,
                    ap=[[0, 128], [1, 512]],
                ),
            )
            nc.vector.tensor_tensor(ctxT[h][:, i0 : i0 + 512], ctx_ps, rzb, op=OP.mult)
            it += 1

    if os.environ.get("SANM_DEBUG", "0") == "1":
        dbg_q = nc.dram_tensor("dbg_q", (H, 128, T), BF16, kind="ExternalOutput").ap()
        dbg_v = nc.dram_tensor("dbg_v", (NC, 128, T), F16, kind="ExternalOutput").ap()
        dbg_c = nc.dram_tensor("dbg_c", (H, 128, T), BF16, kind="ExternalOutput").ap()
        dbg_k = nc.dram_tensor("dbg_k", (H, 128, TK), BF16, kind="ExternalOutput").ap()
        for hh in range(H):
            nc.sync.dma_start(out=dbg_q[hh], in_=qT[hh])
            nc.sync.dma_start(out=dbg_c[hh], in_=ctxT[hh])
            nc.sync.dma_start(out=dbg_k[hh], in_=kTc[hh])
            nc.sync.dma_start(out=dbg_v[hh], in_=vT[hh])

    # ---------------- out projection + fsmn add ----------------
    for tb in range(NT):
        op_ps = ps.tile([128, 512], F32, name="op_ps", tag="s", bufs=4)
        for h in range(H):
            nc.tensor.matmul(
                op_ps,
                ctxT[h][:, tb * 128 : (tb + 1) * 128],
                wo[:, h, :],
                start=(h == 0),
                stop=(h == H - 1),
            )
        # transpose this t-block of fsmn into natural layout (fp16, 1 cyc/row)
        ftp = ps.tile([128, 512], F16, name="ftp", tag="s", bufs=4)
        for c in range(NC):
            nc.tensor.transpose(
                ftp[:, c * 128 : (c + 1) * 128],
                vT[c][:, tb * 128 : (tb + 1) * 128],
                ident_h,
            )
        f_sb = work.tile([128, D], F16, name="f_sb", tag="f_sb", bufs=2)
        nc.scalar.copy(f_sb, ftp)
        o_sb = work.tile([128, D], F32, name="o_sb", tag="o_sb", bufs=2)
        nc.vector.tensor_tensor(o_sb, op_ps, f_sb, op=OP.add)
        nc.sync.dma_start(out=out_d[tb * 128 : (tb + 1) * 128, :], in_=o_sb)

    main_cm.__exit__(None, None, None)
    stack.close()


_CACHE = {}
_FN_CACHE = {}


def make_sharded_fn(nc, n_cores=NCORES):
    """Build a reusable jitted executable for `nc` (done once per build).

    run_bass_kernel_spmd creates a fresh jax.jit per call, so every
    invocation re-traces, re-lowers and re-loads the NEFF; caching the
    jitted callable makes repeat kernel() calls cost only transfer+exec.
    """
    import jax
    from jax.experimental.shard_map import shard_map
    from jax.sharding import Mesh, PartitionSpec

    from concourse import bass2jax
    from concourse.bass2jax import _bass_exec_p, install_neuronx_cc_hook

    install_neuronx_cc_hook()
    partition_name = nc.partition_id_tensor.name if nc.partition_id_tensor else None
    in_names, out_names, out_avals, zero_outs = [], [], [], []
    for alloc in nc.m.functions[0].allocations:
        if not isinstance(alloc, mybir.MemoryLocationSet):
            continue
        name = alloc.memorylocations[0].name
        if alloc.kind == "ExternalInput":
            if name != partition_name:
                in_names.append(name)
        elif alloc.kind == "ExternalOutput":
            out_names.append(name)
            shape = tuple(alloc.tensor_shape)
            dtype = mybir.dt.np(alloc.dtype)
            out_avals.append(jax.core.ShapedArray(shape, dtype))
            zero_outs.append(np.zeros(shape, dtype))
    n_params = len(in_names)
    all_in_names = list(in_names) + list(out_names)
    if partition_name is not None:
        all_in_names.append(partition_name)

    def _body(*args):
        operands = list(args)
        if partition_name is not None:
            operands.append(bass2jax.partition_id_tensor())
        outs = _bass_exec_p.bind(
            *operands,
            out_avals=tuple(out_avals),
            in_names=tuple(all_in_names),
            out_names=tuple(out_names),
            lowering_input_output_aliases=(),
            sim_require_finite=True,
            sim_require_nnan=True,
            nc=nc,
        )
        return tuple(outs)

    devices = jax.devices()[:n_cores]
    mesh = Mesh(np.asarray(devices), ("core",))
    n_outs = len(out_avals)
    in_specs = (PartitionSpec("core"),) * (n_params + n_outs)
    out_specs = (PartitionSpec("core"),) * n_outs
    fn = jax.jit(
        shard_map(
            _body, mesh=mesh, in_specs=in_specs, out_specs=out_specs, check_rep=False
        ),
        keep_unused=True,
    )
    return fn, in_names, out_names, zero_outs


def run_cached(nc, in_maps, key):
    """Execute via a cached jitted executable (falls back to the slow path)."""
    import jax

    if key not in _FN_CACHE:
        _FN_CACHE[key] = make_sharded_fn(nc)
    fn, in_names, out_names, zero_outs = _FN_CACHE[key]
    n = len(in_maps)
    concat_in = [
        np.concatenate([np.asarray(in_maps[c][name]) for c in range(n)], axis=0)
        for name in in_names
    ]
    concat_zeros = [
        np.zeros((n * z.shape[0], *z.shape[1:]), z.dtype) for z in zero_outs
    ]
    out_arrs = fn(*concat_in, *concat_zeros)
    outs = [np.asarray(a) for a in out_arrs]
    return [
        {
            name: outs[i].reshape(n, outs[i].shape[0] // n, *outs[i].shape[1:])[c]
            for i, name in enumerate(out_names)
        }
        for c in range(n)
    ]


def _build(TK):
    key = (QKV_DT, ATT_DT, REPS, TK, NORM, FSMN_DT, LOOP)
    if key in _CACHE:
        return _CACHE[key]
    nc = bacc.Bacc(
        "TRN2",
        target_bir_lowering=False,
        debug=False,
        enable_asserts=False,
        num_devices=NCORES,
    )
    aps = (
        nc.dram_tensor("x", (T, D), BF16, kind="ExternalInput").ap(),
        nc.dram_tensor("mask", (T,), F32, kind="ExternalInput").ap(),
        nc.dram_tensor("xc", (TK, D), BF16, kind="ExternalInput").ap(),
        nc.dram_tensor("cbias", (TK,), F32, kind="ExternalInput").ap(),
        nc.dram_tensor("Wqkv", (D, 3 * D), BF16, kind="ExternalInput").ap(),
        nc.dram_tensor("bqkv", (3 * D,), F32, kind="ExternalInput").ap(),
        nc.dram_tensor("Wout", (D, D), BF16, kind="ExternalInput").ap(),
        nc.dram_tensor("bout", (D,), F32, kind="ExternalInput").ap(),
        nc.dram_tensor("fsmn_w", (D, 1, KS), F32, kind="ExternalInput").ap(),
        nc.dram_tensor("out", (T, D), F32, kind="ExternalOutput").ap(),
    )
    with tile.TileContext(nc) as tc:
        if LOOP > 0:
            # hw loop: NEFF size is constant in trip count, so a large trip
            # count isolates per-rep device time from dispatch overhead
            with tc.For_i(0, LOOP, 1):
                build_kernel_body(tc, aps, TK, 0)
        else:
            for rep in range(REPS):
                build_kernel_body(tc, aps, TK, rep)
    nc.compile()
    _CACHE[key] = nc
    return nc


def _bf16(a):
    import ml_dtypes

    return np.ascontiguousarray(a.astype(ml_dtypes.bfloat16))


def _compact(x_b, mask_b, TK):
    """Host-side gather of unmasked token rows, padded to TK (bf16 in/out)."""
    idx = np.nonzero(mask_b != 0)[0]
    n = len(idx)
    xc = np.zeros((TK, x_b.shape[1]), x_b.dtype)
    xc[:n] = x_b[idx[:TK]]
    cb = np.full((TK,), MASK_NEG, np.float32)
    cb[:n] = 0.0
    return xc, cb


def kernel(x, mask, Wqkv, bqkv, Wout, bout, fsmn_w):
    x = _bf16(np.asarray(x))
    mask = np.ascontiguousarray(np.asarray(mask, dtype=np.float32))
    Wqkv = _bf16(np.asarray(Wqkv))
    bqkv = np.ascontiguousarray(np.asarray(bqkv, dtype=np.float32))
    Wout = _bf16(np.asarray(Wout))
    bout = np.ascontiguousarray(np.asarray(bout, dtype=np.float32))
    fsmn_w = np.ascontiguousarray(np.asarray(fsmn_w, dtype=np.float32))

    counts = [int((mask[b, 0] != 0).sum()) for b in range(NCORES)]
    TK = min(T, max(256, int(-(-max(counts) // 128) * 128)))

    nc = _build(TK)
    in_maps = []
    for b in range(NCORES):
        xc, cb = _compact(x[b], mask[b, 0], TK)
        in_maps.append(
            {
                "x": x[b],
                "mask": np.ascontiguousarray(mask[b, 0]),
                "xc": xc,
                "cbias": cb,
                "Wqkv": Wqkv,
                "bqkv": bqkv,
                "Wout": Wout,
                "bout": bout,
                "fsmn_w": fsmn_w,
            }
        )
    try:
        results = run_cached(nc, in_maps, key=(id(nc), TK))
    except Exception:
        res = bass_utils.run_bass_kernel_spmd(
            nc, in_maps, core_ids=list(range(NCORES)), trace=False
        )
        results = res.results
    out = np.stack([results[b]["out"] for b in range(NCORES)], axis=0)
    return out


if __name__ == "__main__":
    rng = np.random.default_rng(0)
    ins = {
        "x": rng.standard_normal((NCORES, T, D), dtype=np.float32),
        "mask": rng.integers(0, 2, (NCORES, 1, T)).astype(np.float32),
        "Wqkv": (rng.standard_normal((D, 3 * D)) * 0.02).astype(np.float32),
        "bqkv": np.zeros((3 * D,), np.float32),
        "Wout": (rng.standard_normal((D, D)) * 0.02).astype(np.float32),
        "bout": np.zeros((D,), np.float32),
        "fsmn_w": (rng.standard_normal((D, 1, KS)) * 0.1).astype(np.float32),
    }
    out = kernel(**ins)
    print(out.shape, out.dtype, float(np.abs(out).max()))



# revision 54
# speedup vs baseline: 4171.1019x; 1.1270x over previous
"""Trainium2 Bass kernel for MultiHeadedAttentionSANM.

Per-core (data-parallel over batch, 8 cores, B=1 each):
  - qkv^T = (x @ Wqkv)^T on PE (float32r): q^T and full v^T (FSMN needs all
    tokens); k^T and a second v^T are computed only for the ~50% of tokens
    with mask=1, gathered host-side into a compact x_c (TK tokens).
  - FSMN: depthwise conv over time in (d, t) layout on DVE (f32), in place
    on v^T with partial-width taps; mask-muls on GPSIMD.
  - attention: scores computed transposed (compact keys on partitions) so the
    exp output feeds the ctx matmul directly as the rhs stream; masked/padded
    keys get a -30000 exp bias. Softmax denominator Z via a ones-weight PE
    pass; normalization is delayed all the way to the out-proj epilogue
    (per-head PSUM + per-partition 1/Z scalars).
"""

import os
import sys

for _p in ("/opt/trn_rl_repo", "/root/.axon_site/_ro/trn_rl_repo"):
    if os.path.isdir(_p) and _p not in sys.path:
        sys.path.append(_p)

from contextlib import ExitStack

import numpy as np

import concourse.bass as bass
import concourse.mybir as mybir
import concourse.tile as tile
from concourse import bacc
from concourse import bass_utils
from concourse.masks import make_identity

T, D, H, DK, KS, PAD = 2048, 512, 4, 128, 11, 5
NCORES = 8
NT = T // 128          # 16 t-blocks of 128
NC = D // 128          # 4 d-chunks of 128
SCALE = float(DK) ** -0.5
MASK_NEG = -30000.0

F32 = mybir.dt.float32
F32R = mybir.dt.float32r
BF16 = mybir.dt.bfloat16
F16 = mybir.dt.float16
AF = mybir.ActivationFunctionType
OP = mybir.AluOpType

QKV_DT = os.environ.get("SANM_QKV_DT", "f32r")   # f32r | f32
ATT_DT = os.environ.get("SANM_ATT_DT", "bf16")   # bf16 | f32
REPS = int(os.environ.get("SANM_REPS", "1"))     # timing: repeat body in one NEFF
LOOP = int(os.environ.get("SANM_LOOP", "0"))     # timing: hw For_i loop trip count
NORM = os.environ.get("SANM_NORM", "psum")       # psum | inplace
FSMN_DT = os.environ.get("SANM_FSMN_DT", "f32")  # f32 | bf16


def _bcast_vec(ap, nrows):
    """Broadcast a flat [N] DRAM AP across partitions -> [nrows, N]."""
    return bass.AP(tensor=ap.tensor, offset=ap.offset, ap=[[0, nrows]] + list(ap.ap))


def _tiles(total, step=512):
    out, p = [], 0
    while p < total:
        n = min(step, total - p)
        rem = total - p - n
        if 0 < rem < 256:  # avoid <256-wide f32r tails (4 cyc/row penalty)
            n = (n + rem) // 2
            n = (n + 127) // 128 * 128
        out.append((p, n))
        p += n
    return out


def build_kernel_body(tc, aps, TK, rep=0):
    nc = tc.nc
    x_d, mask_d, xc_d, cbias_d, wqkv_d, bqkv_d, wout_d, bout_d, fw_d, out_d = aps
    R = f"r{rep}_" if rep else ""
    TKC = TK // 128  # compact key chunks

    stack = ExitStack()
    consts = stack.enter_context(tc.tile_pool(name=R + "consts", bufs=1))
    work = stack.enter_context(tc.tile_pool(name=R + "work", bufs=2))
    ps = stack.enter_context(tc.tile_pool(name=R + "ps", bufs=1, space="PSUM"))

    # p_main holds all long-lived tensors (whole kernel); p_x nests inside it
    # (LIFO) and is released after the qkv matmuls to reclaim x^T space.
    main_cm = tc.tile_pool(name=R + "p_main", bufs=1)
    x_cm = tc.tile_pool(name=R + "p_x", bufs=1)
    p_main = main_cm.__enter__()
    p_x = x_cm.__enter__()

    # ---------------- constants ----------------
    ident = consts.tile([128, 128], F32, name="ident", tag="ident")
    make_identity(nc, ident)
    ident_r = consts.tile([128, 128], F32R, name="ident_r", tag="ident_r")
    nc.vector.tensor_copy(ident_r, ident.bitcast(F32R))
    ident_b = consts.tile([128, 128], BF16, name="ident_b", tag="ident_b")
    nc.vector.tensor_copy(ident_b, ident)
    ident_h = consts.tile([128, 128], F16, name="ident_h", tag="ident_h")
    nc.vector.tensor_copy(ident_h, ident)

    ones_att = consts.tile([128, 1], BF16, name="ones_att", tag="ones_att")
    nc.vector.memset(ones_att, 1.0)
    ones_row = consts.tile([1, 128], BF16, name="ones_row", tag="ones_row")
    nc.vector.memset(ones_row, 1.0)

    # ---------------- x^T and xc^T (XBAR DMA transposes) ---------------------
    # x/xc arrive bf16 (host pre-converts); the DMA engines' 2-byte transpose
    # mode (14ns per 16x128 tile) replaces the load+PE-transpose+copy pipeline.
    # x first (it gates the v projection -> FSMN chain); consts ride SWDGE.
    xT = p_x.tile([128, NC, T], BF16, name="xT", tag="xT")
    xcT = p_x.tile([128, NC, TK], BF16, name="xcT", tag="xcT")
    for c in range(NC):
        (nc.sync if c % 2 == 0 else nc.scalar).dma_start(
            out=xT[:, c, :], in_=x_d[:, c * 128 : (c + 1) * 128], transpose=True
        )
    for c in range(NC):
        (nc.sync if c % 2 == 0 else nc.scalar).dma_start(
            out=xcT[:, c, :], in_=xc_d[:, c * 128 : (c + 1) * 128], transpose=True
        )

    # compact-key exp bias (0 valid / -30000 padded), as columns (128, TKC)
    mbias = consts.tile([128, TKC], F32, name="mbias", tag="mbias")
    nc.gpsimd.dma_start(out=mbias, in_=cbias_d.rearrange("(c p) -> p c", p=128))

    # mask broadcast across partitions (128, T) bf16 (exact for 0/1), for FSMN
    mrow = consts.tile([128, T], BF16, name="mrow", tag="mrow")
    nc.gpsimd.dma_start(out=mrow, in_=_bcast_vec(mask_d, 128))

    # biases as per-partition columns
    bq = consts.tile([128, 12], F32, name="bq", tag="bq")
    nc.gpsimd.dma_start(out=bq, in_=bqkv_d.rearrange("(c p) -> p c", p=128))
    bo = consts.tile([128, NC], F32, name="bo", tag="bo")
    nc.gpsimd.dma_start(out=bo, in_=bout_d.rearrange("(c p) -> p c", p=128))

    # fsmn weights (128, NC, KS); center tap += 1 (folds the residual)
    wadj = consts.tile([128, NC, KS], F32, name="wadj", tag="wadj")
    nc.gpsimd.dma_start(out=wadj, in_=fw_d.rearrange("(c p) o k -> p c (o k)", p=128))
    nc.vector.tensor_scalar_add(
        wadj[:, :, PAD : PAD + 1], wadj[:, :, PAD : PAD + 1], 1.0
    )

    # Wout (128, NC, 512) bf16, loaded directly (host pre-converts to bf16)
    wo = consts.tile([128, NC, D], BF16, name="wo", tag="wo")
    nc.gpsimd.dma_start(out=wo, in_=wout_d.rearrange("(c p) d -> p c d", p=128))

    # ---------------- qkv^T = (x @ Wqkv)^T ----------------
    # q on full tokens; k only compact; v full (FSMN) and compact (attention)
    qT = [p_main.tile([128, T], BF16, name=f"qT{h}", tag=f"qT{h}") for h in range(H)]
    kTc = [p_main.tile([128, TK], BF16, name=f"kTc{h}", tag=f"kTc{h}") for h in range(H)]
    # fp16 for the FSMN path: same 2-byte DVE speed, 8x finer mantissa (the
    # conv accumulates at the residual's scale, where bf16 rounding is ~1e-2)
    vT = [p_main.tile([128, T], F16, name=f"vT{c}", tag=f"vT{c}") for c in range(NC)]
    vcT = [p_main.tile([128, TK], BF16, name=f"vcT{c}", tag=f"vcT{c}") for c in range(NC)]

    def project(f, srcT, tspans, sink):
        """psum[128, n] = Wqkv[:, f-block].T @ src over d-chunks, then sink.

        dc is the outer loop so consecutive matmuls share one stationary
        (one ldweights per d-chunk instead of per span x chunk)."""
        wqf = work.tile([128, NC, 128], BF16, name="wqf", tag="wqf", bufs=3)
        wqf_src = wqkv_d[:, f * 128 : (f + 1) * 128].rearrange(
            "(c p) f -> p c f", p=128
        )
        nc.gpsimd.dma_start(out=wqf, in_=wqf_src)
        mms = [
            ps.tile([128, 512], F32, name="mmq", tag="s", bufs=4) for _ in tspans
        ]
        for dc in range(NC):
            for i, (t0, n) in enumerate(tspans):
                nc.tensor.matmul(
                    mms[i][:, :n],
                    wqf[:, dc, :],
                    srcT[:, dc, t0 : t0 + n],
                    start=(dc == 0),
                    stop=(dc == NC - 1),
                )
        for i, (t0, n) in enumerate(tspans):
            sink(mms[i], t0, n)

    def act_sink(dst, f):
        def sink(mm, t0, n):
            nc.scalar.activation(
                dst[:, t0 : t0 + n], mm[:, :n], AF.Identity,
                bias=bq[:, f : f + 1], scale=1.0,
            )
        return sink

    # -------- FSMN op stream (d, t layout), fp16, all on DVE -----------------
    # v is projected FIRST so the FSMN conv can run on DVE throughout the
    # PE-heavy q/k/vc projection phase; ops are yielded one at a time and
    # pulled between projection blocks / attention blocks
    vmt = [p_main.tile([128, T], F16, name=f"vmt{c}", tag=f"vmt{c}") for c in range(NC)]
    fac = [p_main.tile([128, T], F16, name=f"fac{c}", tag=f"fac{c}") for c in range(NC)]

    def _fsmn_ops():
        # STT (mult+add) has no fast DVE ucode mode, so each tap is a
        # 4x-mode tensor_scalar mult into a scratch plus a 2x-mode add.
        # Yields one DVE op at a time so the attention loop can interleave
        # them finely and the in-order DVE queue never falls behind.
        for c in range(NC):
            vm, acc = vmt[c], fac[c]
            yield lambda c=c, vm=vm: nc.vector.tensor_tensor(
                vm, vT[c], mrow, op=OP.mult
            )
            yield lambda c=c, vm=vm, acc=acc: nc.vector.tensor_scalar_mul(
                acc, vm, wadj[:, c, PAD : PAD + 1]
            )
            for kk in list(range(0, PAD)) + list(range(PAD + 1, KS)):
                s = kk - PAD
                lo, hi = max(0, -s), T - max(0, s)

                def tapop(c=c, vm=vm, acc=acc, kk=kk, lo=lo, hi=hi, s=s):
                    tap = work.tile([128, T], F16, name="tap", tag="tap", bufs=2)
                    nc.vector.tensor_scalar_mul(
                        tap[:, lo:hi], vm[:, lo + s : hi + s], wadj[:, c, kk : kk + 1]
                    )
                    nc.vector.tensor_tensor(
                        acc[:, lo:hi], acc[:, lo:hi], tap[:, lo:hi], op=OP.add
                    )

                yield tapop
            # out = (conv + vm) * m + bo (center tap carries the +1 residual)
            yield lambda c=c, acc=acc: nc.vector.tensor_tensor(
                vT[c], acc, mrow, op=OP.mult
            )
            yield lambda c=c: nc.vector.tensor_scalar_add(
                vT[c], vT[c], bo[:, c : c + 1]
            )

    fsmn_iter = _fsmn_ops()

    def pull_fsmn(k):
        for _ in range(k):
            op = next(fsmn_iter, None)
            if op is None:
                return
            op()

    for i, f in enumerate(range(8, 12)):  # v full FIRST (unblocks FSMN on DVE)
        project(f, xT, _tiles(T), act_sink(vT[f - 8], f))
        if i > 0:
            pull_fsmn(14)  # chunk i-1 (vT[i-1] is fully sinked by now)
    for f in range(4):  # q: full tokens -> qT (bf16), bias via ACT
        project(f, xT, _tiles(T), act_sink(qT[f], f))
        pull_fsmn(4)
    for f in range(4, 8):  # k: compact tokens -> kTc
        project(f, xcT, _tiles(TK), act_sink(kTc[f - 4], f))
    for f in range(8, 12):  # v compact tokens (attention)
        project(f, xcT, _tiles(TK), act_sink(vcT[f - 8], f))
    x_cm.__exit__(None, None, None)  # frees xT, xcT
    pull_fsmn(99)  # any remainder: epilogue blocks interleave with attention

    # ------------- compact v natural (PE transposes of vcT, batched) ---------
    vh = [
        p_main.tile([128, TKC, 128], BF16, name=f"vh{h}", tag=f"vh{h}")
        for h in range(H)
    ]
    for h in range(H):
        for j0 in range(0, TKC, 4):
            jn = min(4, TKC - j0)
            tp = ps.tile([128, 512], BF16, name="tpv", tag="s", bufs=4)
            for j in range(jn):
                nc.tensor.transpose(
                    tp[:, j * 128 : (j + 1) * 128],
                    vcT[h][:, (j0 + j) * 128 : (j0 + j + 1) * 128],
                    ident_b,
                )
            nc.scalar.copy(vh[h][:, j0 : j0 + jn, :], tp[:, : jn * 128])

    # ---------------- attention + interleaved epilogue ----------------
    # per (query-block of 512, head): scores transposed (compact keys on
    # partitions); exp with -30000 pad bias; ctx accumulates over key chunks
    # in one PSUM bank; Z = sum_k exp via a DVE/Pool add tree + one
    # ones-matmul; 1/Z broadcast across partitions via a DRAM bounce;
    # normalize on DVE. qb is the OUTER loop so each query block's out
    # projection runs as soon as its 4 heads are done (no serial tail).
    ctxT = [
        p_main.tile([128, T], BF16, name=f"ctxT{h}", tag=f"ctxT{h}")
        for h in range(H)
    ]

    def att_block(h, qb, it):
        i0 = qb * 512
        ctx_ps = ps.tile([128, 512], F32, name="ctx_ps", tag="actx", bufs=2)
        esum_d = work.tile([128, 512], BF16, name="esum_d", tag="esum_d", bufs=2)
        for jc in range(TKC):
            s_ps = ps.tile([128, 512], F32, name="s_ps", tag="s", bufs=4)
            nc.tensor.matmul(
                s_ps,
                kTc[h][:, jc * 128 : (jc + 1) * 128],
                qT[h][:, i0 : i0 + 512],
                start=True,
                stop=True,
                skip_group_check=True,
            )
            eT = work.tile([128, 512], BF16, name="eT", tag="eT", bufs=4)
            nc.scalar.activation(
                eT, s_ps, AF.Exp, bias=mbias[:, jc : jc + 1], scale=SCALE
            )
            nc.tensor.matmul(
                ctx_ps,
                vh[h][:, jc, :],
                eT,
                start=(jc == 0),
                stop=(jc == TKC - 1),
                skip_group_check=True,
            )
            # Z = sum_k exp on DVE (bf16 2x mode); attention is ~1% of |out|
            # so bf16 partial sums are plenty
            if jc == 0:
                nc.vector.tensor_copy(esum_d, eT)
            else:
                nc.vector.tensor_tensor(esum_d, esum_d, eT, op=OP.add)
        z_ps = ps.tile([1, 512], F32, name="z_ps", tag="z", bufs=1)
        nc.tensor.matmul(
            z_ps, ones_att, esum_d, start=True, stop=True, skip_group_check=True
        )
        rz = work.tile([1, 512], BF16, name="rz", tag="rz", bufs=2)
        with nc.allow_low_precision(reason="1/Z applied to bf16 attn weights"):
            nc.vector.reciprocal(rz, z_ps)
        # broadcast 1/Z across partitions as a PE outer product (ones x rz)
        zb_ps = ps.tile([128, 512], F32, name="zb_ps", tag="zb", bufs=1)
        nc.tensor.matmul(
            zb_ps, ones_row, rz, start=True, stop=True, skip_group_check=True
        )
        zb_sb = work.tile([128, 512], BF16, name="zb_sb", tag="zb_sb", bufs=2)
        nc.scalar.copy(zb_sb, zb_ps)
        nc.vector.tensor_tensor(ctxT[h][:, i0 : i0 + 512], ctx_ps, zb_sb, op=OP.mult)

    def out_block(tb):
        op_ps = ps.tile([128, 512], F32, name="op_ps", tag="actx", bufs=2)
        for h in range(H):
            nc.tensor.matmul(
                op_ps,
                ctxT[h][:, tb * 128 : (tb + 1) * 128],
                wo[:, h, :],
                start=(h == 0),
                stop=(h == H - 1),
            )
        # transpose this t-block of fsmn into natural layout (fp16, 1 cyc/row)
        ftp = ps.tile([128, 512], F16, name="ftp", tag="zb", bufs=1)
        for c in range(NC):
            nc.tensor.transpose(
                ftp[:, c * 128 : (c + 1) * 128],
                vT[c][:, tb * 128 : (tb + 1) * 128],
                ident_h,
            )
        f_sb = work.tile([128, D], F16, name="f_sb", tag="f_sb", bufs=2)
        nc.scalar.copy(f_sb, ftp)
        o_sb = work.tile([128, D], F32, name="o_sb", tag="o_sb", bufs=2)
        nc.vector.tensor_tensor(o_sb, op_ps, f_sb, op=OP.add)
        nc.sync.dma_start(out=out_d[tb * 128 : (tb + 1) * 128, :], in_=o_sb)

    it = 0
    for qb in range(4):  # query blocks of 512
        for h in range(H):
            att_block(h, qb, it)
            it += 1
        if os.environ.get("SANM_EPI", "end") == "interleave":
            for tb4 in range(4):  # this query block's 128-token out blocks
                out_block(qb * 4 + tb4)
    if os.environ.get("SANM_EPI", "end") != "interleave":
        for tb in range(NT):
            out_block(tb)

    if os.environ.get("SANM_DEBUG", "0") == "1":
        dbg_q = nc.dram_tensor("dbg_q", (H, 128, T), BF16, kind="ExternalOutput").ap()
        dbg_v = nc.dram_tensor("dbg_v", (NC, 128, T), F16, kind="ExternalOutput").ap()
        dbg_c = nc.dram_tensor("dbg_c", (H, 128, T), BF16, kind="ExternalOutput").ap()
        dbg_k = nc.dram_tensor("dbg_k", (H, 128, TK), BF16, kind="ExternalOutput").ap()
        for hh in range(H):
            nc.sync.dma_start(out=dbg_q[hh], in_=qT[hh])
            nc.sync.dma_start(out=dbg_c[hh], in_=ctxT[hh])
            nc.sync.dma_start(out=dbg_k[hh], in_=kTc[hh])
            nc.sync.dma_start(out=dbg_v[hh], in_=vT[hh])

    main_cm.__exit__(None, None, None)
    stack.close()


_CACHE = {}
_FN_CACHE = {}


def make_sharded_fn(nc, n_cores=NCORES):
    """Build a reusable jitted executable for `nc` (done once per build).

    run_bass_kernel_spmd creates a fresh jax.jit per call, so every
    invocation re-traces, re-lowers and re-loads the NEFF; caching the
    jitted callable makes repeat kernel() calls cost only transfer+exec.
    """
    import jax
    from jax.experimental.shard_map import shard_map
    from jax.sharding import Mesh, PartitionSpec

    from concourse import bass2jax
    from concourse.bass2jax import _bass_exec_p, install_neuronx_cc_hook

    install_neuronx_cc_hook()
    partition_name = nc.partition_id_tensor.name if nc.partition_id_tensor else None
    in_names, out_names, out_avals, zero_outs = [], [], [], []
    for alloc in nc.m.functions[0].allocations:
        if not isinstance(alloc, mybir.MemoryLocationSet):
            continue
        name = alloc.memorylocations[0].name
        if alloc.kind == "ExternalInput":
            if name != partition_name:
                in_names.append(name)
        elif alloc.kind == "ExternalOutput":
            out_names.append(name)
            shape = tuple(alloc.tensor_shape)
            dtype = mybir.dt.np(alloc.dtype)
            out_avals.append(jax.core.ShapedArray(shape, dtype))
            zero_outs.append(np.zeros(shape, dtype))
    n_params = len(in_names)
    all_in_names = list(in_names) + list(out_names)
    if partition_name is not None:
        all_in_names.append(partition_name)

    def _body(*args):
        operands = list(args)
        if partition_name is not None:
            operands.append(bass2jax.partition_id_tensor())
        outs = _bass_exec_p.bind(
            *operands,
            out_avals=tuple(out_avals),
            in_names=tuple(all_in_names),
            out_names=tuple(out_names),
            lowering_input_output_aliases=(),
            sim_require_finite=True,
            sim_require_nnan=True,
            nc=nc,
        )
        return tuple(outs)

    devices = jax.devices()[:n_cores]
    mesh = Mesh(np.asarray(devices), ("core",))
    n_outs = len(out_avals)
    in_specs = (PartitionSpec("core"),) * (n_params + n_outs)
    out_specs = (PartitionSpec("core"),) * n_outs
    fn = jax.jit(
        shard_map(
            _body, mesh=mesh, in_specs=in_specs, out_specs=out_specs, check_rep=False
        ),
        keep_unused=True,
    )
    return fn, in_names, out_names, zero_outs


def run_cached(nc, in_maps, key):
    """Execute via a cached jitted executable (falls back to the slow path)."""
    import jax

    if key not in _FN_CACHE:
        _FN_CACHE[key] = make_sharded_fn(nc)
    fn, in_names, out_names, zero_outs = _FN_CACHE[key]
    n = len(in_maps)
    concat_in = [
        np.concatenate([np.asarray(in_maps[c][name]) for c in range(n)], axis=0)
        for name in in_names
    ]
    concat_zeros = [
        np.zeros((n * z.shape[0], *z.shape[1:]), z.dtype) for z in zero_outs
    ]
    out_arrs = fn(*concat_in, *concat_zeros)
    outs = [np.asarray(a) for a in out_arrs]
    return [
        {
            name: outs[i].reshape(n, outs[i].shape[0] // n, *outs[i].shape[1:])[c]
            for i, name in enumerate(out_names)
        }
        for c in range(n)
    ]


def _build(TK):
    key = (QKV_DT, ATT_DT, REPS, TK, NORM, FSMN_DT, LOOP)
    if key in _CACHE:
        return _CACHE[key]
    nc = bacc.Bacc(
        "TRN2",
        target_bir_lowering=False,
        debug=False,
        enable_asserts=False,
        num_devices=NCORES,
    )
    aps = (
        nc.dram_tensor("x", (T, D), BF16, kind="ExternalInput").ap(),
        nc.dram_tensor("mask", (T,), F32, kind="ExternalInput").ap(),
        nc.dram_tensor("xc", (TK, D), BF16, kind="ExternalInput").ap(),
        nc.dram_tensor("cbias", (TK,), F32, kind="ExternalInput").ap(),
        nc.dram_tensor("Wqkv", (D, 3 * D), BF16, kind="ExternalInput").ap(),
        nc.dram_tensor("bqkv", (3 * D,), F32, kind="ExternalInput").ap(),
        nc.dram_tensor("Wout", (D, D), BF16, kind="ExternalInput").ap(),
        nc.dram_tensor("bout", (D,), F32, kind="ExternalInput").ap(),
        nc.dram_tensor("fsmn_w", (D, 1, KS), F32, kind="ExternalInput").ap(),
        nc.dram_tensor("out", (T, D), F32, kind="ExternalOutput").ap(),
    )
    with tile.TileContext(nc) as tc:
        if LOOP > 0:
            # hw loop: NEFF size is constant in trip count, so a large trip
            # count isolates per-rep device time from dispatch overhead
            with tc.For_i(0, LOOP, 1):
                build_kernel_body(tc, aps, TK, 0)
        else:
            for rep in range(REPS):
                build_kernel_body(tc, aps, TK, rep)
    nc.compile()
    _CACHE[key] = nc
    return nc


def _bf16(a):
    import ml_dtypes

    return np.ascontiguousarray(a.astype(ml_dtypes.bfloat16))


def _compact(x_b, mask_b, TK):
    """Host-side gather of unmasked token rows, padded to TK (bf16 in/out)."""
    idx = np.nonzero(mask_b != 0)[0]
    n = len(idx)
    xc = np.zeros((TK, x_b.shape[1]), x_b.dtype)
    xc[:n] = x_b[idx[:TK]]
    cb = np.full((TK,), MASK_NEG, np.float32)
    cb[:n] = 0.0
    return xc, cb


def kernel(x, mask, Wqkv, bqkv, Wout, bout, fsmn_w):
    x = _bf16(np.asarray(x))
    mask = np.ascontiguousarray(np.asarray(mask, dtype=np.float32))
    Wqkv = _bf16(np.asarray(Wqkv))
    bqkv = np.ascontiguousarray(np.asarray(bqkv, dtype=np.float32))
    Wout = _bf16(np.asarray(Wout))
    bout = np.ascontiguousarray(np.asarray(bout, dtype=np.float32))
    fsmn_w = np.ascontiguousarray(np.asarray(fsmn_w, dtype=np.float32))

    counts = [int((mask[b, 0] != 0).sum()) for b in range(NCORES)]
    TK = min(T, max(256, int(-(-max(counts) // 128) * 128)))

    nc = _build(TK)
    in_maps = []
    for b in range(NCORES):
        xc, cb = _compact(x[b], mask[b, 0], TK)
        in_maps.append(
            {
                "x": x[b],
                "mask": np.ascontiguousarray(mask[b, 0]),
                "xc": xc,
                "cbias": cb,
                "Wqkv": Wqkv,
                "bqkv": bqkv,
                "Wout": Wout,
                "bout": bout,
                "fsmn_w": fsmn_w,
            }
        )
    try:
        results = run_cached(nc, in_maps, key=(id(nc), TK))
    except Exception:
        res = bass_utils.run_bass_kernel_spmd(
            nc, in_maps, core_ids=list(range(NCORES)), trace=False
        )
        results = res.results
    out = np.stack([results[b]["out"] for b in range(NCORES)], axis=0)
    return out


if __name__ == "__main__":
    rng = np.random.default_rng(0)
    ins = {
        "x": rng.standard_normal((NCORES, T, D), dtype=np.float32),
        "mask": rng.integers(0, 2, (NCORES, 1, T)).astype(np.float32),
        "Wqkv": (rng.standard_normal((D, 3 * D)) * 0.02).astype(np.float32),
        "bqkv": np.zeros((3 * D,), np.float32),
        "Wout": (rng.standard_normal((D, D)) * 0.02).astype(np.float32),
        "bout": np.zeros((D,), np.float32),
        "fsmn_w": (rng.standard_normal((D, 1, KS)) * 0.1).astype(np.float32),
    }
    out = kernel(**ins)
    print(out.shape, out.dtype, float(np.abs(out).max()))



# revision 55
# speedup vs baseline: 4242.9695x; 1.0172x over previous
"""Trainium2 Bass kernel for MultiHeadedAttentionSANM.

Per-core (data-parallel over batch, 8 cores, B=1 each):
  - qkv^T = (x @ Wqkv)^T on PE (float32r): q^T and full v^T (FSMN needs all
    tokens); k^T and a second v^T are computed only for the ~50% of tokens
    with mask=1, gathered host-side into a compact x_c (TK tokens).
  - FSMN: depthwise conv over time in (d, t) layout on DVE (f32), in place
    on v^T with partial-width taps; mask-muls on GPSIMD.
  - attention: scores computed transposed (compact keys on partitions) so the
    exp output feeds the ctx matmul directly as the rhs stream; masked/padded
    keys get a -30000 exp bias. Softmax denominator Z via a ones-weight PE
    pass; normalization is delayed all the way to the out-proj epilogue
    (per-head PSUM + per-partition 1/Z scalars).
"""

import os
import sys

for _p in ("/opt/trn_rl_repo", "/root/.axon_site/_ro/trn_rl_repo"):
    if os.path.isdir(_p) and _p not in sys.path:
        sys.path.append(_p)

from contextlib import ExitStack

import numpy as np

import concourse.bass as bass
import concourse.mybir as mybir
import concourse.tile as tile
from concourse import bacc
from concourse import bass_utils
from concourse.masks import make_identity

T, D, H, DK, KS, PAD = 2048, 512, 4, 128, 11, 5
NCORES = 8
NT = T // 128          # 16 t-blocks of 128
NC = D // 128          # 4 d-chunks of 128
SCALE = float(DK) ** -0.5
MASK_NEG = -30000.0

F32 = mybir.dt.float32
F32R = mybir.dt.float32r
BF16 = mybir.dt.bfloat16
F16 = mybir.dt.float16
AF = mybir.ActivationFunctionType
OP = mybir.AluOpType

QKV_DT = os.environ.get("SANM_QKV_DT", "f32r")   # f32r | f32
ATT_DT = os.environ.get("SANM_ATT_DT", "bf16")   # bf16 | f32
REPS = int(os.environ.get("SANM_REPS", "1"))     # timing: repeat body in one NEFF
LOOP = int(os.environ.get("SANM_LOOP", "0"))     # timing: hw For_i loop trip count
NORM = os.environ.get("SANM_NORM", "psum")       # psum | inplace
FSMN_DT = os.environ.get("SANM_FSMN_DT", "f32")  # f32 | bf16


def _bcast_vec(ap, nrows):
    """Broadcast a flat [N] DRAM AP across partitions -> [nrows, N]."""
    return bass.AP(tensor=ap.tensor, offset=ap.offset, ap=[[0, nrows]] + list(ap.ap))


def _tiles(total, step=512):
    out, p = [], 0
    while p < total:
        n = min(step, total - p)
        rem = total - p - n
        if 0 < rem < 256:  # avoid <256-wide f32r tails (4 cyc/row penalty)
            n = (n + rem) // 2
            n = (n + 127) // 128 * 128
        out.append((p, n))
        p += n
    return out


def build_kernel_body(tc, aps, TK, rep=0):
    nc = tc.nc
    x_d, mask_d, xc_d, cbias_d, wqkv_d, bqkv_d, wout_d, bout_d, fw_d, out_d = aps
    R = f"r{rep}_" if rep else ""
    TKC = TK // 128  # compact key chunks

    stack = ExitStack()
    consts = stack.enter_context(tc.tile_pool(name=R + "consts", bufs=1))
    work = stack.enter_context(tc.tile_pool(name=R + "work", bufs=2))
    ps = stack.enter_context(tc.tile_pool(name=R + "ps", bufs=1, space="PSUM"))

    # p_main holds all long-lived tensors (whole kernel); p_x nests inside it
    # (LIFO) and is released after the qkv matmuls to reclaim x^T space.
    main_cm = tc.tile_pool(name=R + "p_main", bufs=1)
    x_cm = tc.tile_pool(name=R + "p_x", bufs=1)
    p_main = main_cm.__enter__()
    p_x = x_cm.__enter__()

    # ---------------- constants ----------------
    ident = consts.tile([128, 128], F32, name="ident", tag="ident")
    make_identity(nc, ident)
    ident_r = consts.tile([128, 128], F32R, name="ident_r", tag="ident_r")
    nc.vector.tensor_copy(ident_r, ident.bitcast(F32R))
    ident_b = consts.tile([128, 128], BF16, name="ident_b", tag="ident_b")
    nc.vector.tensor_copy(ident_b, ident)
    ident_h = consts.tile([128, 128], F16, name="ident_h", tag="ident_h")
    nc.vector.tensor_copy(ident_h, ident)

    ones_att = consts.tile([128, 1], BF16, name="ones_att", tag="ones_att")
    nc.vector.memset(ones_att, 1.0)
    ones_row = consts.tile([1, 128], BF16, name="ones_row", tag="ones_row")
    nc.vector.memset(ones_row, 1.0)

    # ---------------- x^T and xc^T (XBAR DMA transposes) ---------------------
    # x/xc arrive bf16 (host pre-converts); the DMA engines' 2-byte transpose
    # mode (14ns per 16x128 tile) replaces the load+PE-transpose+copy pipeline.
    # x first (it gates the v projection -> FSMN chain); consts ride SWDGE.
    xT = p_x.tile([128, NC, T], BF16, name="xT", tag="xT")
    xcT = p_x.tile([128, NC, TK], BF16, name="xcT", tag="xcT")
    for c in range(NC):
        (nc.sync if c % 2 == 0 else nc.scalar).dma_start(
            out=xT[:, c, :], in_=x_d[:, c * 128 : (c + 1) * 128], transpose=True
        )
    for c in range(NC):
        (nc.sync if c % 2 == 0 else nc.scalar).dma_start(
            out=xcT[:, c, :], in_=xc_d[:, c * 128 : (c + 1) * 128], transpose=True
        )

    # compact-key exp bias (0 valid / -30000 padded), as columns (128, TKC)
    mbias = consts.tile([128, TKC], F32, name="mbias", tag="mbias")
    nc.gpsimd.dma_start(out=mbias, in_=cbias_d.rearrange("(c p) -> p c", p=128))

    # mask broadcast across partitions (128, T) bf16 (exact for 0/1), for FSMN
    mrow = consts.tile([128, T], BF16, name="mrow", tag="mrow")
    nc.gpsimd.dma_start(out=mrow, in_=_bcast_vec(mask_d, 128))

    # biases as per-partition columns
    bq = consts.tile([128, 12], F32, name="bq", tag="bq")
    nc.gpsimd.dma_start(out=bq, in_=bqkv_d.rearrange("(c p) -> p c", p=128))
    bo = consts.tile([128, NC], F32, name="bo", tag="bo")
    nc.gpsimd.dma_start(out=bo, in_=bout_d.rearrange("(c p) -> p c", p=128))

    # fsmn weights (128, NC, KS); center tap += 1 (folds the residual)
    wadj = consts.tile([128, NC, KS], F32, name="wadj", tag="wadj")
    nc.gpsimd.dma_start(out=wadj, in_=fw_d.rearrange("(c p) o k -> p c (o k)", p=128))
    nc.vector.tensor_scalar_add(
        wadj[:, :, PAD : PAD + 1], wadj[:, :, PAD : PAD + 1], 1.0
    )

    # Wout (128, NC, 512) bf16, loaded directly (host pre-converts to bf16)
    wo = consts.tile([128, NC, D], BF16, name="wo", tag="wo")
    nc.gpsimd.dma_start(out=wo, in_=wout_d.rearrange("(c p) d -> p c d", p=128))

    # ---------------- qkv^T = (x @ Wqkv)^T ----------------
    # q on full tokens; k only compact; v full (FSMN) and compact (attention)
    qT = [p_main.tile([128, T], BF16, name=f"qT{h}", tag=f"qT{h}") for h in range(H)]
    kTc = [p_main.tile([128, TK], BF16, name=f"kTc{h}", tag=f"kTc{h}") for h in range(H)]
    # fp16 for the FSMN path: same 2-byte DVE speed, 8x finer mantissa (the
    # conv accumulates at the residual's scale, where bf16 rounding is ~1e-2)
    vT = [p_main.tile([128, T], F16, name=f"vT{c}", tag=f"vT{c}") for c in range(NC)]
    vcT = [p_main.tile([128, TK], BF16, name=f"vcT{c}", tag=f"vcT{c}") for c in range(NC)]

    def project(f, srcT, tspans, sink):
        """psum[128, n] = Wqkv[:, f-block].T @ src over d-chunks, then sink.

        dc is the outer loop so consecutive matmuls share one stationary
        (one ldweights per d-chunk instead of per span x chunk)."""
        wqf = work.tile([128, NC, 128], BF16, name="wqf", tag="wqf", bufs=3)
        wqf_src = wqkv_d[:, f * 128 : (f + 1) * 128].rearrange(
            "(c p) f -> p c f", p=128
        )
        nc.gpsimd.dma_start(out=wqf, in_=wqf_src)
        mms = [
            ps.tile([128, 512], F32, name="mmq", tag="s", bufs=4) for _ in tspans
        ]
        for dc in range(NC):
            for i, (t0, n) in enumerate(tspans):
                nc.tensor.matmul(
                    mms[i][:, :n],
                    wqf[:, dc, :],
                    srcT[:, dc, t0 : t0 + n],
                    start=(dc == 0),
                    stop=(dc == NC - 1),
                )
        for i, (t0, n) in enumerate(tspans):
            sink(mms[i], t0, n)

    def act_sink(dst, f):
        def sink(mm, t0, n):
            nc.scalar.activation(
                dst[:, t0 : t0 + n], mm[:, :n], AF.Identity,
                bias=bq[:, f : f + 1], scale=1.0,
            )
        return sink

    # -------- FSMN op stream (d, t layout), fp16, all on DVE -----------------
    # v is projected FIRST so the FSMN conv can run on DVE throughout the
    # PE-heavy q/k/vc projection phase; ops are yielded one at a time and
    # pulled between projection blocks / attention blocks
    vmt = [p_main.tile([128, T], F16, name=f"vmt{c}", tag=f"vmt{c}") for c in range(NC)]
    fac = [p_main.tile([128, T], F16, name=f"fac{c}", tag=f"fac{c}") for c in range(NC)]

    def _fsmn_ops():
        # STT (mult+add) has no fast DVE ucode mode, so each tap is a
        # 4x-mode tensor_scalar mult into a scratch plus a 2x-mode add.
        # Yields one DVE op at a time so the attention loop can interleave
        # them finely and the in-order DVE queue never falls behind.
        for c in range(NC):
            vm, acc = vmt[c], fac[c]
            yield lambda c=c, vm=vm: nc.vector.tensor_tensor(
                vm, vT[c], mrow, op=OP.mult
            )
            yield lambda c=c, vm=vm, acc=acc: nc.vector.tensor_scalar_mul(
                acc, vm, wadj[:, c, PAD : PAD + 1]
            )
            for kk in list(range(0, PAD)) + list(range(PAD + 1, KS)):
                s = kk - PAD
                lo, hi = max(0, -s), T - max(0, s)

                def tapop(c=c, vm=vm, acc=acc, kk=kk, lo=lo, hi=hi, s=s):
                    tap = work.tile([128, T], F16, name="tap", tag="tap", bufs=2)
                    nc.vector.tensor_scalar_mul(
                        tap[:, lo:hi], vm[:, lo + s : hi + s], wadj[:, c, kk : kk + 1]
                    )
                    nc.vector.tensor_tensor(
                        acc[:, lo:hi], acc[:, lo:hi], tap[:, lo:hi], op=OP.add
                    )

                yield tapop
            # out = (conv + vm) * m + bo (center tap carries the +1 residual)
            yield lambda c=c, acc=acc: nc.vector.tensor_tensor(
                vT[c], acc, mrow, op=OP.mult
            )
            yield lambda c=c: nc.vector.tensor_scalar_add(
                vT[c], vT[c], bo[:, c : c + 1]
            )

    fsmn_iter = _fsmn_ops()

    def pull_fsmn(k):
        for _ in range(k):
            op = next(fsmn_iter, None)
            if op is None:
                return
            op()

    for i, f in enumerate(range(8, 12)):  # v full FIRST (unblocks FSMN on DVE)
        project(f, xT, _tiles(T), act_sink(vT[f - 8], f))
        if i > 0:
            pull_fsmn(14)  # chunk i-1 (vT[i-1] is fully sinked by now)
    for f in range(4):  # q: full tokens -> qT (bf16), bias via ACT
        project(f, xT, _tiles(T), act_sink(qT[f], f))
        pull_fsmn(4)
    for f in range(4, 8):  # k: compact tokens -> kTc
        project(f, xcT, _tiles(TK), act_sink(kTc[f - 4], f))
    for f in range(8, 12):  # v compact tokens (attention)
        project(f, xcT, _tiles(TK), act_sink(vcT[f - 8], f))
    x_cm.__exit__(None, None, None)  # frees xT, xcT
    pull_fsmn(99)  # any remainder: epilogue blocks interleave with attention

    # ------------- compact v natural (PE transposes of vcT, batched) ---------
    vh = [
        p_main.tile([128, TKC, 128], BF16, name=f"vh{h}", tag=f"vh{h}")
        for h in range(H)
    ]
    for h in range(H):
        for j0 in range(0, TKC, 4):
            jn = min(4, TKC - j0)
            tp = ps.tile([128, 512], BF16, name="tpv", tag="s", bufs=4)
            for j in range(jn):
                nc.tensor.transpose(
                    tp[:, j * 128 : (j + 1) * 128],
                    vcT[h][:, (j0 + j) * 128 : (j0 + j + 1) * 128],
                    ident_b,
                )
            nc.scalar.copy(vh[h][:, j0 : j0 + jn, :], tp[:, : jn * 128])

    # ---------------- attention + interleaved epilogue ----------------
    # per (query-block of 512, head): scores transposed (compact keys on
    # partitions); exp with -30000 pad bias; ctx accumulates over key chunks
    # in one PSUM bank; Z = sum_k exp via a DVE/Pool add tree + one
    # ones-matmul; 1/Z broadcast across partitions via a DRAM bounce;
    # normalize on DVE. qb is the OUTER loop so each query block's out
    # projection runs as soon as its 4 heads are done (no serial tail).
    ctxT = [
        p_main.tile([128, T], BF16, name=f"ctxT{h}", tag=f"ctxT{h}")
        for h in range(H)
    ]

    def att_block(h, qb, it):
        i0 = qb * 512
        ctx_ps = ps.tile([128, 512], F32, name="ctx_ps", tag="actx", bufs=3)
        esum_d = work.tile([128, 512], BF16, name="esum_d", tag="esum_d", bufs=2)
        for jc in range(TKC):
            s_ps = ps.tile([128, 512], F32, name="s_ps", tag="s", bufs=4)
            nc.tensor.matmul(
                s_ps,
                kTc[h][:, jc * 128 : (jc + 1) * 128],
                qT[h][:, i0 : i0 + 512],
                start=True,
                stop=True,
                skip_group_check=True,
            )
            eT = work.tile([128, 512], BF16, name="eT", tag="eT", bufs=4)
            nc.scalar.activation(
                eT, s_ps, AF.Exp, bias=mbias[:, jc : jc + 1], scale=SCALE
            )
            nc.tensor.matmul(
                ctx_ps,
                vh[h][:, jc, :],
                eT,
                start=(jc == 0),
                stop=(jc == TKC - 1),
                skip_group_check=True,
            )
            # Z = sum_k exp on DVE (bf16 2x mode); attention is ~1% of |out|
            # so bf16 partial sums are plenty
            if jc == 0:
                nc.vector.tensor_copy(esum_d, eT)
            else:
                nc.vector.tensor_tensor(esum_d, esum_d, eT, op=OP.add)
        z_ps = ps.tile([1, 512], F32, name="z_ps", tag="z", bufs=1)
        nc.tensor.matmul(
            z_ps, ones_att, esum_d, start=True, stop=True, skip_group_check=True
        )
        rz = work.tile([1, 512], BF16, name="rz", tag="rz", bufs=2)
        with nc.allow_low_precision(reason="1/Z applied to bf16 attn weights"):
            nc.vector.reciprocal(rz, z_ps)
        # broadcast 1/Z across partitions as a PE outer product (ones x rz)
        zb_ps = ps.tile([128, 512], F32, name="zb_ps", tag="zb", bufs=1)
        nc.tensor.matmul(
            zb_ps, ones_row, rz, start=True, stop=True, skip_group_check=True
        )
        zb_sb = work.tile([128, 512], BF16, name="zb_sb", tag="zb_sb", bufs=2)
        nc.scalar.copy(zb_sb, zb_ps)
        nc.vector.tensor_tensor(ctxT[h][:, i0 : i0 + 512], ctx_ps, zb_sb, op=OP.mult)

    def out_block(tb):
        op_ps = ps.tile([128, 512], F32, name="op_ps", tag="actx", bufs=2)
        for h in range(H):
            nc.tensor.matmul(
                op_ps,
                ctxT[h][:, tb * 128 : (tb + 1) * 128],
                wo[:, h, :],
                start=(h == 0),
                stop=(h == H - 1),
            )
        # transpose this t-block of fsmn into natural layout (fp16, 1 cyc/row)
        ftp = ps.tile([128, 512], F16, name="ftp", tag="zb", bufs=1)
        for c in range(NC):
            nc.tensor.transpose(
                ftp[:, c * 128 : (c + 1) * 128],
                vT[c][:, tb * 128 : (tb + 1) * 128],
                ident_h,
            )
        f_sb = work.tile([128, D], F16, name="f_sb", tag="f_sb", bufs=2)
        nc.scalar.copy(f_sb, ftp)
        o_sb = work.tile([128, D], F32, name="o_sb", tag="o_sb", bufs=2)
        nc.vector.tensor_tensor(o_sb, op_ps, f_sb, op=OP.add)
        nc.sync.dma_start(out=out_d[tb * 128 : (tb + 1) * 128, :], in_=o_sb)

    it = 0
    for qb in range(4):  # query blocks of 512
        for h in range(H):
            att_block(h, qb, it)
            it += 1
        if os.environ.get("SANM_EPI", "end") == "interleave":
            for tb4 in range(4):  # this query block's 128-token out blocks
                out_block(qb * 4 + tb4)
    if os.environ.get("SANM_EPI", "end") != "interleave":
        for tb in range(NT):
            out_block(tb)

    if os.environ.get("SANM_DEBUG", "0") == "1":
        dbg_q = nc.dram_tensor("dbg_q", (H, 128, T), BF16, kind="ExternalOutput").ap()
        dbg_v = nc.dram_tensor("dbg_v", (NC, 128, T), F16, kind="ExternalOutput").ap()
        dbg_c = nc.dram_tensor("dbg_c", (H, 128, T), BF16, kind="ExternalOutput").ap()
        dbg_k = nc.dram_tensor("dbg_k", (H, 128, TK), BF16, kind="ExternalOutput").ap()
        for hh in range(H):
            nc.sync.dma_start(out=dbg_q[hh], in_=qT[hh])
            nc.sync.dma_start(out=dbg_c[hh], in_=ctxT[hh])
            nc.sync.dma_start(out=dbg_k[hh], in_=kTc[hh])
            nc.sync.dma_start(out=dbg_v[hh], in_=vT[hh])

    main_cm.__exit__(None, None, None)
    stack.close()


_CACHE = {}
_FN_CACHE = {}


def make_sharded_fn(nc, n_cores=NCORES):
    """Build a reusable jitted executable for `nc` (done once per build).

    run_bass_kernel_spmd creates a fresh jax.jit per call, so every
    invocation re-traces, re-lowers and re-loads the NEFF; caching the
    jitted callable makes repeat kernel() calls cost only transfer+exec.
    """
    import jax
    from jax.experimental.shard_map import shard_map
    from jax.sharding import Mesh, PartitionSpec

    from concourse import bass2jax
    from concourse.bass2jax import _bass_exec_p, install_neuronx_cc_hook

    install_neuronx_cc_hook()
    partition_name = nc.partition_id_tensor.name if nc.partition_id_tensor else None
    in_names, out_names, out_avals, zero_outs = [], [], [], []
    for alloc in nc.m.functions[0].allocations:
        if not isinstance(alloc, mybir.MemoryLocationSet):
            continue
        name = alloc.memorylocations[0].name
        if alloc.kind == "ExternalInput":
            if name != partition_name:
                in_names.append(name)
        elif alloc.kind == "ExternalOutput":
            out_names.append(name)
            shape = tuple(alloc.tensor_shape)
            dtype = mybir.dt.np(alloc.dtype)
            out_avals.append(jax.core.ShapedArray(shape, dtype))
            zero_outs.append(np.zeros(shape, dtype))
    n_params = len(in_names)
    all_in_names = list(in_names) + list(out_names)
    if partition_name is not None:
        all_in_names.append(partition_name)

    def _body(*args):
        operands = list(args)
        if partition_name is not None:
            operands.append(bass2jax.partition_id_tensor())
        outs = _bass_exec_p.bind(
            *operands,
            out_avals=tuple(out_avals),
            in_names=tuple(all_in_names),
            out_names=tuple(out_names),
            lowering_input_output_aliases=(),
            sim_require_finite=True,
            sim_require_nnan=True,
            nc=nc,
        )
        return tuple(outs)

    devices = jax.devices()[:n_cores]
    mesh = Mesh(np.asarray(devices), ("core",))
    n_outs = len(out_avals)
    in_specs = (PartitionSpec("core"),) * (n_params + n_outs)
    out_specs = (PartitionSpec("core"),) * n_outs
    fn = jax.jit(
        shard_map(
            _body, mesh=mesh, in_specs=in_specs, out_specs=out_specs, check_rep=False
        ),
        keep_unused=True,
    )
    return fn, in_names, out_names, zero_outs


def run_cached(nc, in_maps, key):
    """Execute via a cached jitted executable (falls back to the slow path)."""
    import jax

    if key not in _FN_CACHE:
        _FN_CACHE[key] = make_sharded_fn(nc)
    fn, in_names, out_names, zero_outs = _FN_CACHE[key]
    n = len(in_maps)
    concat_in = [
        np.concatenate([np.asarray(in_maps[c][name]) for c in range(n)], axis=0)
        for name in in_names
    ]
    concat_zeros = [
        np.zeros((n * z.shape[0], *z.shape[1:]), z.dtype) for z in zero_outs
    ]
    out_arrs = fn(*concat_in, *concat_zeros)
    outs = [np.asarray(a) for a in out_arrs]
    return [
        {
            name: outs[i].reshape(n, outs[i].shape[0] // n, *outs[i].shape[1:])[c]
            for i, name in enumerate(out_names)
        }
        for c in range(n)
    ]


def _build(TK):
    key = (QKV_DT, ATT_DT, REPS, TK, NORM, FSMN_DT, LOOP)
    if key in _CACHE:
        return _CACHE[key]
    nc = bacc.Bacc(
        "TRN2",
        target_bir_lowering=False,
        debug=False,
        enable_asserts=False,
        num_devices=NCORES,
    )
    aps = (
        nc.dram_tensor("x", (T, D), BF16, kind="ExternalInput").ap(),
        nc.dram_tensor("mask", (T,), F32, kind="ExternalInput").ap(),
        nc.dram_tensor("xc", (TK, D), BF16, kind="ExternalInput").ap(),
        nc.dram_tensor("cbias", (TK,), F32, kind="ExternalInput").ap(),
        nc.dram_tensor("Wqkv", (D, 3 * D), BF16, kind="ExternalInput").ap(),
        nc.dram_tensor("bqkv", (3 * D,), F32, kind="ExternalInput").ap(),
        nc.dram_tensor("Wout", (D, D), BF16, kind="ExternalInput").ap(),
        nc.dram_tensor("bout", (D,), F32, kind="ExternalInput").ap(),
        nc.dram_tensor("fsmn_w", (D, 1, KS), F32, kind="ExternalInput").ap(),
        nc.dram_tensor("out", (T, D), F32, kind="ExternalOutput").ap(),
    )
    with tile.TileContext(nc) as tc:
        if LOOP > 0:
            # hw loop: NEFF size is constant in trip count, so a large trip
            # count isolates per-rep device time from dispatch overhead
            with tc.For_i(0, LOOP, 1):
                build_kernel_body(tc, aps, TK, 0)
        else:
            for rep in range(REPS):
                build_kernel_body(tc, aps, TK, rep)
    nc.compile()
    _CACHE[key] = nc
    return nc


def _bf16(a):
    import ml_dtypes

    return np.ascontiguousarray(a.astype(ml_dtypes.bfloat16))


def _compact(x_b, mask_b, TK):
    """Host-side gather of unmasked token rows, padded to TK (bf16 in/out)."""
    idx = np.nonzero(mask_b != 0)[0]
    n = len(idx)
    xc = np.zeros((TK, x_b.shape[1]), x_b.dtype)
    xc[:n] = x_b[idx[:TK]]
    cb = np.full((TK,), MASK_NEG, np.float32)
    cb[:n] = 0.0
    return xc, cb


def kernel(x, mask, Wqkv, bqkv, Wout, bout, fsmn_w):
    x = _bf16(np.asarray(x))
    mask = np.ascontiguousarray(np.asarray(mask, dtype=np.float32))
    Wqkv = _bf16(np.asarray(Wqkv))
    bqkv = np.ascontiguousarray(np.asarray(bqkv, dtype=np.float32))
    Wout = _bf16(np.asarray(Wout))
    bout = np.ascontiguousarray(np.asarray(bout, dtype=np.float32))
    fsmn_w = np.ascontiguousarray(np.asarray(fsmn_w, dtype=np.float32))

    counts = [int((mask[b, 0] != 0).sum()) for b in range(NCORES)]
    TK = min(T, max(256, int(-(-max(counts) // 128) * 128)))

    nc = _build(TK)
    in_maps = []
    for b in range(NCORES):
        xc, cb = _compact(x[b], mask[b, 0], TK)
        in_maps.append(
            {
                "x": x[b],
                "mask": np.ascontiguousarray(mask[b, 0]),
                "xc": xc,
                "cbias": cb,
                "Wqkv": Wqkv,
                "bqkv": bqkv,
                "Wout": Wout,
                "bout": bout,
                "fsmn_w": fsmn_w,
            }
        )
    try:
        results = run_cached(nc, in_maps, key=(id(nc), TK))
    except Exception:
        res = bass_utils.run_bass_kernel_spmd(
            nc, in_maps, core_ids=list(range(NCORES)), trace=False
        )
        results = res.results
    out = np.stack([results[b]["out"] for b in range(NCORES)], axis=0)
    return out


if __name__ == "__main__":
    rng = np.random.default_rng(0)
    ins = {
        "x": rng.standard_normal((NCORES, T, D), dtype=np.float32),
        "mask": rng.integers(0, 2, (NCORES, 1, T)).astype(np.float32),
        "Wqkv": (rng.standard_normal((D, 3 * D)) * 0.02).astype(np.float32),
        "bqkv": np.zeros((3 * D,), np.float32),
        "Wout": (rng.standard_normal((D, D)) * 0.02).astype(np.float32),
        "bout": np.zeros((D,), np.float32),
        "fsmn_w": (rng.standard_normal((D, 1, KS)) * 0.1).astype(np.float32),
    }
    out = kernel(**ins)
    print(out.shape, out.dtype, float(np.abs(out).max()))

